# revision 1
# baseline (speedup 1.0000x reference)
"""GATConvBlock (GAT attention + BatchNorm + LeakyReLU) on 8 Trainium2
NeuronCores. Self-contained: host-side edge scheduling + Bass/Tile program +
SPMD execution via concourse.

Strategy: dst-sharded graph parallelism. Each core owns 12500 dst nodes; x is
rotated per core so its shard is local rows 0:12500. Per-core table
[h(bf16)|a_src,a_dst(f32)] built on device (x@W via f32r matmuls), per-edge
rows fetched with int16 dma_gather over 4 source ranges, segment softmax +
aggregation via membership matmuls in PSUM, BatchNorm stats AllReduce across
cores, normalize + LeakyReLU on device.
"""
import concourse.tile as _tile_mod
"""GATConv block kernel for Trainium2, 8 NeuronCores, dst-sharded.

Each core c receives x rotated by -c*shard so its own dst shard is table rows
0:shard; gather indices are (src - c*shard) mod N. One identical program runs
SPMD on all 8 cores.
"""
import numpy as np
import ml_dtypes

import concourse.bass as bass
import concourse.bacc as bacc
import concourse.tile as tile
from concourse import mybir
from concourse.bass_utils import run_bass_kernel_spmd

# ---- walrus compat: split multi-wait sync_info (this toolchain rejects >1) ----
from concourse import mybir as _mb
from concourse.tile import TileContext as _TC
from concourse.vector_clock import ScopedClock as _SC

_MAX_WAITS = 1


def _patched_drain_and_barrier(self, tick_clock, wait_clock):
    drain_inst = self.nc.sync.drain()
    wait_clock.add_sem_waits(drain_inst.ins, _SC({None: tick_clock.global_clock}))
    si = drain_inst.ins.sync_info
    waits = list(si.on_wait or [])
    if len(waits) > _MAX_WAITS:
        si.on_wait = waits[:_MAX_WAITS]
        for w in waits[_MAX_WAITS:]:
            n = self.nc.sync.nop()
            n.ins.sync_info = _mb.SyncInfo(on_wait=[w], on_update=[])
        self.nc.sync.drain()
    self.nc.all_engine_barrier()
    popped = self.nc._tile_sem_poison_stack.pop()
    assert popped is self._sem_poison
    self.nc.clear_and_free_semaphores(list(self.sems.allocated().values()))
    self.nc.all_engine_barrier()


_orig_commit = _TC._commit_instruction


def _patched_commit(self, inst, lazy_reg_writes=True):
    si = inst.sync_info
    if si is not None and si.on_wait and len(si.on_wait) > _MAX_WAITS and \
            inst.engine != _mb.EngineType.Unassigned:
        waits = list(si.on_wait)
        si.on_wait = waits[:_MAX_WAITS]
        eng = self.nc.engines[inst.engine]
        for w in waits[_MAX_WAITS:]:
            n = eng.nop()
            n.ins.sync_info = _mb.SyncInfo(on_wait=[w], on_update=[])
    return _orig_commit(self, inst, lazy_reg_writes)


if getattr(_TC, "_gat_patched", False) is False:
    _TC._drain_and_barrier = _patched_drain_and_barrier
    _TC._commit_instruction = _patched_commit
    _TC._gat_patched = True
# ---- end walrus compat ----


f32 = mybir.dt.float32
f32r = mybir.dt.float32r
bf16 = mybir.dt.bfloat16
i16 = mybir.dt.int16
i32 = mybir.dt.int32
AF = mybir.ActivationFunctionType
ALU = mybir.AluOpType

P = 128
RANGE = 32768
NEG = 0.2
BN_EPS = 1e-5


def build_schedule(src_g, dst_g, n_nodes, n_cores, tiles_per_sc=4):
    shard = n_nodes // n_cores
    sc_width = tiles_per_sc * P
    n_sc = (shard + sc_width - 1) // sc_width
    n_ranges = (n_nodes + RANGE - 1) // RANGE
    n_tiles = (shard + P - 1) // P

    order = np.argsort(dst_g, kind="stable")
    src_s = src_g[order]
    dst_s = dst_g[order]
    core_bounds = np.searchsorted(dst_s, np.arange(n_cores + 1) * shard)

    counts = np.zeros((n_cores, n_sc, n_ranges), np.int64)
    segs = [[[None] * n_ranges for _ in range(n_sc)] for _ in range(n_cores)]
    for c in range(n_cores):
        lo, hi = core_bounds[c], core_bounds[c + 1]
        s = (src_s[lo:hi] - c * shard) % n_nodes   # rotated source index
        d = dst_s[lo:hi] - c * shard               # local dst 0..shard
        sc_id = d // sc_width
        r_id = s // RANGE
        o2 = np.lexsort((r_id, sc_id))
        s, d, sc_id, r_id = s[o2], d[o2], sc_id[o2], r_id[o2]
        key = sc_id * n_ranges + r_id
        cnt = np.bincount(key, minlength=n_sc * n_ranges).reshape(n_sc, n_ranges)
        counts[c] = cnt
        offs = np.concatenate([[0], np.cumsum(cnt.ravel())])
        for sc in range(n_sc):
            for r in range(n_ranges):
                k = sc * n_ranges + r
                sl = slice(offs[k], offs[k + 1])
                segs[c][sc][r] = (s[sl], d[sl])

    caps = counts.max(axis=0)
    caps = ((caps + P - 1) // P) * P

    targets = {}
    for sc in range(n_sc):
        for r in range(n_ranges):
            for k in range(int(caps[sc, r]) // P):
                tg = set()
                for c in range(n_cores):
                    dseg = segs[c][sc][r][1]
                    part = dseg[k * P:(k + 1) * P]
                    if len(part):
                        tg |= set(np.unique(part // P).tolist())
                targets[(sc, r, k)] = sorted(tg)

    covered = set()
    for v in targets.values():
        covered.update(v)
    for t in range(n_tiles):
        if t not in covered:
            sc = t // tiles_per_sc
            for r in range(n_ranges):
                if caps[sc, r] > 0:
                    targets[(sc, r, 0)] = sorted(set(targets[(sc, r, 0)]) | {t})
                    break

    o16 = np.zeros((n_sc, n_ranges), np.int64)
    odl = np.zeros((n_sc, n_ranges), np.int64)
    acc16 = accdl = 0
    for sc in range(n_sc):
        for r in range(n_ranges):
            o16[sc, r] = acc16
            odl[sc, r] = accdl
            acc16 += int(caps[sc, r]) // 16
            accdl += int(caps[sc, r]) // P
    idxcols, dlcols = acc16, accdl

    per_core = []
    for c in range(n_cores):
        idx_arr = np.zeros((16, idxcols), np.int16)
        dl_arr = np.full((P, dlcols), -1.0, np.float32)
        for sc in range(n_sc):
            for r in range(n_ranges):
                cap = int(caps[sc, r])
                if cap == 0:
                    continue
                sseg, dseg = segs[c][sc][r]
                n = len(sseg)
                i16v = np.zeros(cap, np.int16)
                i16v[:n] = (sseg - r * RANGE).astype(np.int16)
                dlv = np.full(cap, -1.0, np.float32)
                dlv[:n] = (dseg - sc * sc_width).astype(np.float32)
                idx_arr[:, int(o16[sc, r]): int(o16[sc, r]) + cap // 16] = \
                    i16v.reshape(cap // 16, 16).T
                dl_arr[:, int(odl[sc, r]): int(odl[sc, r]) + cap // P] = \
                    dlv.reshape(cap // P, P).T
        per_core.append({"g_idx": np.tile(idx_arr, (8, 1)), "g_dstloc": dl_arr})

    return {
        "n_sc": n_sc, "n_ranges": n_ranges, "caps": caps, "o16": o16,
        "odl": odl, "idxcols": idxcols, "dlcols": dlcols, "targets": targets,
        "shard": shard, "n_tiles": n_tiles, "tiles_per_sc": tiles_per_sc,
        "sc_width": sc_width, "n_nodes": n_nodes,
    }, per_core


def build_program(sched, heads=4, head_dim=32, msg_dbf16=False, phases=3):
    n_nodes = sched["n_nodes"]
    in_ch = P
    out_ch = heads * head_dim
    shard = sched["shard"]
    n_sc, n_ranges = sched["n_sc"], sched["n_ranges"]
    caps, o16, odl = sched["caps"], sched["o16"], sched["odl"]
    targets = sched["targets"]
    n_tiles = sched["n_tiles"]
    tiles_per_sc = sched["tiles_per_sc"]
    sc_width = sched["sc_width"]
    H2 = 2 * heads

    NAGG = (256 + 8) if msg_dbf16 else (128 + 4)

    nc = bacc.Bacc("TRN2", target_bir_lowering=False, num_devices=8, num_swdge_queues=4)
    x_in = nc.dram_tensor("x", [n_nodes, in_ch], f32, kind="ExternalInput")
    w_in = nc.dram_tensor("w", [in_ch, out_ch], f32, kind="ExternalInput")
    asrc_in = nc.dram_tensor("att_src", [heads, head_dim], f32, kind="ExternalInput")
    adst_in = nc.dram_tensor("att_dst", [heads, head_dim], f32, kind="ExternalInput")
    gamma_in = nc.dram_tensor("gamma", [1, out_ch], f32, kind="ExternalInput")
    beta_in = nc.dram_tensor("beta", [1, out_ch], f32, kind="ExternalInput")
    gidx_in = nc.dram_tensor("g_idx", [P, sched["idxcols"]], i16, kind="ExternalInput")
    gdl_in = nc.dram_tensor("g_dstloc", [P, sched["dlcols"]], f32,
                            kind="ExternalInput")
    ident_in = nc.dram_tensor("ident", [P, P], f32, kind="ExternalInput")
    iota_in = nc.dram_tensor("iota", [P, sc_width], f32, kind="ExternalInput")
    out_fin = nc.dram_tensor("out", [shard, out_ch], f32, kind="ExternalOutput")

    with tile.TileContext(nc) as tc:
        with tc.tile_pool(name="dram", bufs=1, space="DRAM") as drp, \
             tc.tile_pool(name="const", bufs=1) as cs:
            table = drp.tile([n_nodes, in_ch], f32)
            out_pre = drp.tile([shard, out_ch], f32)
            bn_in = drp.tile([1, 2 * out_ch], f32)
            bn_out = drp.tile([1, 2 * out_ch], f32)

            # ---------------- constants ----------------
            identf = cs.tile([P, P], f32)
            nc.sync.dma_start(out=identf[:], in_=ident_in[:])
            identr = cs.tile([P, P], f32r)
            nc.vector.tensor_copy(out=identr[:], in_=identf[:])
            identb = cs.tile([P, P], bf16)
            nc.vector.tensor_copy(out=identb[:], in_=identf[:])
            iota_f = cs.tile([P, sc_width], f32)
            nc.sync.dma_start(out=iota_f[:], in_=iota_in[:])
            alpha_c = cs.tile([P, 1], f32)
            nc.vector.memset(alpha_c[:], NEG)
            ones_bf = cs.tile([P, 1], bf16)
            nc.vector.memset(ones_bf[:], 1.0)
            ones1f = cs.tile([1, P], f32)
            nc.vector.memset(ones1f[:], 1.0)
            ones1r = cs.tile([1, P], f32r)
            nc.vector.tensor_copy(out=ones1r[:], in_=ones1f[:])
            eps_c = cs.tile([1, 1], f32)
            nc.vector.memset(eps_c[:], BN_EPS)

            gidx_sb = cs.tile([P, sched["idxcols"]], i16)
            nc.sync.dma_start(out=gidx_sb[:], in_=gidx_in[:])
            gdl_sb = cs.tile([P, sched["dlcols"]], f32)
            nc.sync.dma_start(out=gdl_sb[:], in_=gdl_in[:])

            with tc.tile_pool(name="ph0ps", bufs=2, space="PSUM") as psc:
                A_sb = cs.tile([P, H2], f32)
                nc.vector.memset(A_sb[:], 0.0)
                for hd in range(heads):
                    nc.sync.dma_start(
                        out=A_sb[hd * head_dim:(hd + 1) * head_dim, hd:hd + 1],
                        in_=asrc_in[hd:hd + 1, :].rearrange("a b -> b a"))
                    nc.sync.dma_start(
                        out=A_sb[hd * head_dim:(hd + 1) * head_dim,
                                 heads + hd:heads + hd + 1],
                        in_=adst_in[hd:hd + 1, :].rearrange("a b -> b a"))
                Ar = cs.tile([P, H2], f32r)
                nc.vector.tensor_copy(out=Ar[:], in_=A_sb[:])
                Wr = cs.tile([P, P], f32r)
                nc.sync.dma_start(out=Wr[:], in_=w_in[:].bitcast(f32r))
                WT_ps = psc.tile([P, P], f32r)
                nc.tensor.transpose(out=WT_ps[:], in_=Wr[:], identity=identr[:])
                WT = cs.tile([P, P], f32r)
                nc.scalar.copy(out=WT[:], in_=WT_ps[:])
                WA_ps = psc.tile([P, H2], f32)
                nc.tensor.matmul(out=WA_ps[:], lhsT=WT[:], rhs=Ar[:],
                                 start=True, stop=True)
                zpad = cs.tile([P, 256 - P - H2], f32)
                nc.vector.memset(zpad[:], 0.0)
                rhs_big = cs.tile([P, 256], f32r)
                nc.vector.tensor_copy(out=rhs_big[:, P + H2:256], in_=zpad[:])
                nc.vector.tensor_copy(out=rhs_big[:, 0:P], in_=Wr[:])
                nc.vector.tensor_copy(out=rhs_big[:, P:P + H2], in_=WA_ps[:])

            # ---------------- phase 1: table ----------------
            with tc.tile_pool(name="p1", bufs=3) as p1, \
                 tc.tile_pool(name="p1ps", bufs=2, space="PSUM") as p1ps, \
                 tc.tile_pool(name="p1ps2", bufs=2, space="PSUM") as p1ps2:
                n_nt = (n_nodes + P - 1) // P
                for nt in range(n_nt):
                    r0 = nt * P
                    r1 = min(r0 + P, n_nodes)
                    pt = r1 - r0
                    xr = p1.tile([P, P], f32r, tag="xr")
                    nc.sync.dma_start(out=xr[:pt], in_=x_in[r0:r1, :].bitcast(f32r))
                    xT_ps = p1ps.tile([P, P], f32r, tag="xtp")
                    nc.tensor.transpose(out=xT_ps[:, :pt], in_=xr[:pt],
                                        identity=identr[:pt, :pt])
                    xT = p1.tile([P, P], f32r, tag="xt")
                    nc.scalar.copy(out=xT[:, :pt], in_=xT_ps[:, :pt])
                    h_ps = p1ps2.tile([P, 256], f32, tag="hp")
                    nc.tensor.matmul(out=h_ps[:pt], lhsT=xT[:, :pt],
                                     rhs=rhs_big[:], start=True, stop=True)
                    hbf = p1.tile([P, P], bf16, tag="hbf")
                    nc.vector.tensor_copy(out=hbf[:pt], in_=h_ps[:pt, 0:P])
                    sc8 = p1.tile([P, H2], f32, tag="sc8")
                    nc.scalar.copy(out=sc8[:pt], in_=h_ps[:pt, P:P + H2])
                    nc.sync.dma_start(
                        out=table[r0:r1, 0:in_ch // 2].bitcast(bf16),
                        in_=hbf[:pt])
                    nc.sync.dma_start(
                        out=table[r0:r1, 64:64 + H2], in_=sc8[:pt])

            # ---------------- phase 2 ----------------
            stats_acc = cs.tile([1, 2 * out_ch], f32)
            if phases < 2:
                with tc.tile_pool(name="cp", bufs=2) as cp:
                    for t in range(n_tiles):
                        d0, d1 = t * P, min(t * P + P, shard)
                        pt = d1 - d0
                        tt = cp.tile([P, out_ch], f32, tag="cp", name=f"cp{t}")
                        nc.sync.dma_start(out=tt[:pt], in_=table[d0:d1, :])
                        nc.sync.dma_start(out=out_fin[d0:d1, :], in_=tt[:pt])
                skip_rest = True
            else:
                skip_rest = False
            nc.vector.memset(stats_acc[:], 0.0)

            if not skip_rest:
                gq = [0]
                with tc.tile_pool(name="gb", bufs=2) as gbp, \
                     tc.tile_pool(name="wk", bufs=4) as wk, \
                     tc.tile_pool(name="tl", bufs=2 * tiles_per_sc) as tlp, \
                     tc.tile_pool(name="agg", bufs=tiles_per_sc, space="PSUM") as aggp, \
                     tc.tile_pool(name="trp", bufs=2, space="PSUM") as trp, \
                     tc.tile_pool(name="aep", bufs=1, space="PSUM") as aep, \
                     tc.tile_pool(name="smp", bufs=1, space="PSUM") as smp:
                    for sc in range(n_sc):
                        t_lo = sc * tiles_per_sc
                        t_hi = min(t_lo + tiles_per_sc, n_tiles)

                        adst_hl = {}
                        psums = {}
                        chunk_lists = {}
                        for t in range(t_lo, t_hi):
                            d0, d1 = t * P, min(t * P + P, shard)
                            pt = d1 - d0
                            adf = tlp.tile([P, heads], f32, tag="adf")
                            nc.sync.dma_start(
                                out=adf[:pt],
                                in_=table[d0:d1, 64 + heads:64 + H2])
                            ahl = tlp.tile([P, H2], bf16, tag="ahl")
                            nc.vector.tensor_copy(out=ahl[:pt, 0:heads], in_=adf[:pt])
                            hi_f = tlp.tile([P, heads], f32, tag="hif")
                            nc.vector.tensor_copy(out=hi_f[:pt], in_=ahl[:pt, 0:heads])
                            lo_f = tlp.tile([P, heads], f32, tag="lof")
                            nc.vector.tensor_tensor(out=lo_f[:pt], in0=adf[:pt],
                                                    in1=hi_f[:pt], op=ALU.subtract)
                            nc.vector.tensor_copy(out=ahl[:pt, heads:H2], in_=lo_f[:pt])
                            adst_hl[t] = ahl
                            psums[t] = aggp.tile([P, NAGG], f32, tag="agg", name=f"agg_ps_{t}")
                            chunk_lists[t] = [
                                (r, k)
                                for r in range(n_ranges)
                                for k in range(int(caps[sc, r]) // P)
                                if t in targets[(sc, r, k)]]

                        gbufs = {}
                        for r in range(n_ranges):
                            cap = int(caps[sc, r])
                            if cap == 0:
                                continue
                            cols = cap // P
                            g = gbp.tile([P, cols, P], f32, tag=f"g{r}", name=f"gbuf_{sc}_{r}")
                            base = r * RANGE
                            end = min(base + RANGE, n_nodes)
                            off = 0
                            while off < cap:
                                piece = min(896, cap - off)
                                nc.gpsimd.dma_gather(
                                    out_ap=g[:, off // P: (off + piece + P - 1) // P, :],
                                    in_ap=table[base:end, :],
                                    idxs_ap=gidx_sb[:, int(o16[sc, r]) + off // 16:
                                                    int(o16[sc, r]) + (off + piece) // 16],
                                    num_idxs=piece,
                                    num_idxs_reg=piece,
                                    elem_size=in_ch,
                                    queue_num=gq[0] % 4,
                                )
                                gq[0] += 1
                                off += piece
                            gbufs[r] = g

                        done = {t: 0 for t in range(t_lo, t_hi)}
                        for r in range(n_ranges):
                            cap = int(caps[sc, r])
                            for k in range(cap // P):
                                tgs = targets[(sc, r, k)]
                                if not tgs:
                                    continue
                                rows = gbufs[r][:, k, :]
                                h_view = rows.bitcast(bf16)[:, 0:P]
                                asrc_v = rows[:, 64:64 + heads]
                                dl_col = gdl_sb[:, int(odl[sc, r]) + k:
                                                int(odl[sc, r]) + k + 1]
                                for t in tgs:
                                    tl = t - t_lo
                                    Mc = wk.tile([P, P], bf16, tag="mc")
                                    nc.vector.tensor_scalar(
                                        out=Mc[:], in0=iota_f[:, tl * P:(tl + 1) * P],
                                        scalar1=dl_col, scalar2=None,
                                        op0=ALU.is_equal)
                                    McT_ps = trp.tile([P, P], bf16, tag="tr")
                                    nc.tensor.transpose(out=McT_ps[:], in_=Mc[:],
                                                        identity=identb[:])
                                    McT = wk.tile([P, P], bf16, tag="mct")
                                    nc.scalar.copy(out=McT[:], in_=McT_ps[:])
                                    ae_ps = aep.tile([P, H2], f32, tag="ae")
                                    nc.tensor.matmul(out=ae_ps[:], lhsT=McT[:],
                                                     rhs=adst_hl[t][:],
                                                     start=True, stop=True)
                                    s_t = wk.tile([P, heads], f32, tag="s")
                                    nc.vector.tensor_tensor(
                                        out=s_t[:], in0=asrc_v,
                                        in1=ae_ps[:, 0:heads], op=ALU.add)
                                    nc.vector.tensor_tensor(
                                        out=s_t[:], in0=s_t[:],
                                        in1=ae_ps[:, heads:H2], op=ALU.add)
                                    lr = wk.tile([P, heads], f32, tag="lr")
                                    nc.scalar.activation(out=lr[:], in_=s_t[:],
                                                         func=AF.Prelu,
                                                         alpha=alpha_c[:, 0:1])
                                    me = wk.tile([P, NAGG], bf16, tag="me")
                                    if not msg_dbf16:
                                        nc.scalar.activation(out=me[:, 128:132],
                                                             in_=lr[:], func=AF.Exp)
                                        exv = me[:, 128:132]
                                        ex_b = bass.AP(
                                            tensor=exv.tensor, offset=exv.offset,
                                            ap=[exv.ap[0], exv.ap[1], [0, 32]])
                                        nc.vector.tensor_tensor(
                                            out=me[:, 0:128].rearrange(
                                                "p (h c) -> p h c", h=heads),
                                            in0=h_view.rearrange(
                                                "p (h c) -> p h c", h=heads),
                                            in1=ex_b, op=ALU.mult)
                                    else:
                                        # msg hi/lo + ex hi/lo
                                        mf = wk.tile([P, P], f32, tag="mf")
                                        for hd in range(heads):
                                            nc.vector.tensor_scalar(
                                                out=mf[:, hd * 32:(hd + 1) * 32],
                                                in0=h_view[:, hd * 32:(hd + 1) * 32],
                                                scalar1=ex_f[:, hd:hd + 1],
                                                scalar2=None, op0=ALU.mult)
                                        nc.vector.tensor_copy(out=me[:, 0:P], in_=mf[:])
                                        mhi = wk.tile([P, P], f32, tag="mhi")
                                        nc.vector.tensor_copy(out=mhi[:], in_=me[:, 0:P])
                                        mlo = wk.tile([P, P], f32, tag="mlo")
                                        nc.vector.tensor_tensor(out=mlo[:], in0=mf[:],
                                                                in1=mhi[:],
                                                                op=ALU.subtract)
                                        nc.vector.tensor_copy(out=me[:, P:2 * P],
                                                              in_=mlo[:])
                                        nc.vector.tensor_copy(out=me[:, 256:260],
                                                              in_=ex_f[:])
                                        ehi_f = wk.tile([P, heads], f32, tag="ehf")
                                        nc.vector.tensor_copy(out=ehi_f[:],
                                                              in_=me[:, 256:260])
                                        elo = wk.tile([P, heads], f32, tag="elo")
                                        nc.vector.tensor_tensor(out=elo[:], in0=ex_f[:],
                                                                in1=ehi_f[:],
                                                                op=ALU.subtract)
                                        nc.vector.tensor_copy(out=me[:, 260:264],
                                                              in_=elo[:])
                                    first = done[t] == 0
                                    last = done[t] == len(chunk_lists[t]) - 1
                                    nc.tensor.matmul(out=psums[t][:], lhsT=Mc[:],
                                                     rhs=me[:], start=first, stop=last)
                                    done[t] += 1

                        # ---- per-tile epilogue ----
                        for t in range(t_lo, t_hi):
                            d0, d1 = t * P, min(t * P + P, shard)
                            pt = d1 - d0
                            ps = psums[t]
                            den = tlp.tile([P, heads], f32, tag="den")
                            rden = tlp.tile([P, heads], f32, tag="rden")
                            if not msg_dbf16:
                                nc.vector.tensor_scalar(
                                    out=den[:pt], in0=ps[:pt, 128:132], scalar1=1e-16,
                                    scalar2=None, op0=ALU.add)
                            else:
                                nc.vector.tensor_tensor(
                                    out=den[:pt], in0=ps[:pt, 256:260],
                                    in1=ps[:pt, 260:264], op=ALU.add)
                                nc.vector.tensor_scalar(
                                    out=den[:pt], in0=den[:pt], scalar1=1e-16,
                                    scalar2=None, op0=ALU.add)
                            nc.vector.reciprocal(out=rden[:pt], in_=den[:pt])
                            o_t = tlp.tile([P, P], f32, tag="o")
                            if msg_dbf16:
                                osum = tlp.tile([P, P], f32, tag="osum")
                                nc.vector.tensor_tensor(out=osum[:pt],
                                                        in0=ps[:pt, 0:P],
                                                        in1=ps[:pt, P:2 * P],
                                                        op=ALU.add)
                                src_ps = osum
                            else:
                                src_ps = ps
                            rv = rden[:pt]
                            rd_b = bass.AP(tensor=rv.tensor, offset=rv.offset,
                                           ap=[rv.ap[0], rv.ap[1], [0, 32]])
                            nc.vector.tensor_tensor(
                                out=o_t[:pt].rearrange("p (h c) -> p h c", h=heads),
                                in0=src_ps[:pt, 0:128].rearrange(
                                    "p (h c) -> p h c", h=heads),
                                in1=rd_b, op=ALU.mult)
                            nc.sync.dma_start(out=out_pre[d0:d1, :], in_=o_t[:pt])
                            st_in = tlp.tile([P, 2 * P], bf16, tag="stin")
                            nc.vector.tensor_copy(out=st_in[:pt, 0:P], in_=o_t[:pt])
                            nc.vector.tensor_tensor(out=st_in[:pt, P:2 * P],
                                                    in0=o_t[:pt], in1=o_t[:pt],
                                                    op=ALU.mult)
                            sm_ps = smp.tile([1, 2 * P], f32, tag="sm")
                            nc.tensor.matmul(out=sm_ps[:], lhsT=ones_bf[:pt],
                                             rhs=st_in[:pt], start=True, stop=True)
                            nc.vector.tensor_tensor(out=stats_acc[:],
                                                    in0=stats_acc[:], in1=sm_ps[:],
                                                    op=ALU.add)

            if (not skip_rest) and phases >= 3:
                # ---------------- phase 2.5: BN stats allreduce ----------------
                with tc.tile_pool(name="bn", bufs=1) as bnp, \
                     tc.tile_pool(name="bnps", bufs=1, space="PSUM") as bnps:
                    nc.sync.dma_start(out=bn_in[:], in_=stats_acc[:])
                    nc.gpsimd.collective_compute(
                        "AllReduce", ALU.add,
                        replica_groups=[list(range(8))],
                        ins=[bn_in[:]], outs=[bn_out[:]],
                    )
                    tots = bnp.tile([1, 2 * out_ch], f32)
                    nc.sync.dma_start(out=tots[:], in_=bn_out[:])
                    gam = bnp.tile([1, out_ch], f32)
                    nc.sync.dma_start(out=gam[:], in_=gamma_in[:])
                    bet = bnp.tile([1, out_ch], f32)
                    nc.sync.dma_start(out=bet[:], in_=beta_in[:])
                    mu = bnp.tile([1, out_ch], f32)
                    nc.vector.tensor_scalar(out=mu[:], in0=tots[:, 0:out_ch],
                                            scalar1=1.0 / n_nodes, scalar2=None,
                                            op0=ALU.mult)
                    ex2 = bnp.tile([1, out_ch], f32)
                    nc.vector.tensor_scalar(out=ex2[:], in0=tots[:, out_ch:],
                                            scalar1=1.0 / n_nodes, scalar2=None,
                                            op0=ALU.mult)
                    musq = bnp.tile([1, out_ch], f32)
                    nc.vector.tensor_tensor(out=musq[:], in0=mu[:], in1=mu[:],
                                            op=ALU.mult)
                    var = bnp.tile([1, out_ch], f32)
                    nc.vector.tensor_tensor(out=var[:], in0=ex2[:], in1=musq[:],
                                            op=ALU.subtract)
                    sd = bnp.tile([1, out_ch], f32)
                    nc.scalar.activation(out=sd[:], in_=var[:], func=AF.Sqrt,
                                         bias=eps_c[0:1, 0:1], scale=1.0)
                    rs = bnp.tile([1, out_ch], f32)
                    nc.vector.reciprocal(out=rs[:], in_=sd[:])
                    scale_r = bnp.tile([1, out_ch], f32)
                    nc.vector.tensor_tensor(out=scale_r[:], in0=rs[:], in1=gam[:],
                                            op=ALU.mult)
                    mshift = bnp.tile([1, out_ch], f32)
                    nc.vector.tensor_tensor(out=mshift[:], in0=mu[:], in1=scale_r[:],
                                            op=ALU.mult)
                    shift_r = bnp.tile([1, out_ch], f32)
                    nc.vector.tensor_tensor(out=shift_r[:], in0=bet[:], in1=mshift[:],
                                            op=ALU.subtract)
                    scsh = bnp.tile([1, 2 * out_ch], f32r)
                    nc.vector.tensor_copy(out=scsh[:, 0:out_ch], in_=scale_r[:])
                    nc.vector.tensor_copy(out=scsh[:, out_ch:], in_=shift_r[:])
                    bc_ps = bnps.tile([P, 2 * out_ch], f32)
                    nc.tensor.matmul(out=bc_ps[:], lhsT=ones1r[:], rhs=scsh[:],
                                     start=True, stop=True)
                    scale_bc = cs.tile([P, out_ch], f32)
                    nc.vector.tensor_copy(out=scale_bc[:], in_=bc_ps[:, 0:out_ch])
                    shift_bc = cs.tile([P, out_ch], f32)
                    nc.scalar.copy(out=shift_bc[:], in_=bc_ps[:, out_ch:])

                # ---------------- phase 3: normalize ----------------
                with tc.tile_pool(name="p3", bufs=3) as p3:
                    for t in range(n_tiles):
                        d0, d1 = t * P, min(t * P + P, shard)
                        pt = d1 - d0
                        o_t = p3.tile([P, out_ch], f32, tag="o3")
                        nc.sync.dma_start(out=o_t[:pt], in_=out_pre[d0:d1, :])
                        y_t = p3.tile([P, out_ch], f32, tag="y3")
                        nc.vector.tensor_tensor(out=y_t[:pt], in0=o_t[:pt],
                                                in1=scale_bc[:pt], op=ALU.mult)
                        nc.vector.tensor_tensor(out=y_t[:pt], in0=y_t[:pt],
                                                in1=shift_bc[:pt], op=ALU.add)
                        z_t = p3.tile([P, out_ch], f32, tag="z3")
                        nc.scalar.activation(out=z_t[:pt], in_=y_t[:pt],
                                             func=AF.Prelu, alpha=alpha_c[:pt, 0:1])
                        nc.sync.dma_start(out=out_fin[d0:d1, :], in_=z_t[:pt])

            if (not skip_rest) and phases < 3:
                with tc.tile_pool(name="cp2", bufs=2) as cp2:
                    for t in range(n_tiles):
                        d0, d1 = t * P, min(t * P + P, shard)
                        pt = d1 - d0
                        tt2 = cp2.tile([P, out_ch], f32, tag="cp2", name=f"cq{t}")
                        nc.sync.dma_start(out=tt2[:pt], in_=out_pre[d0:d1, :])
                        nc.sync.dma_start(out=out_fin[d0:d1, :], in_=tt2[:pt])

    nc.compile()
    return nc


def run(x, edge_index, W, att_src, att_dst, bias, gamma, beta,
        n_cores=8, tiles_per_sc=4, msg_dbf16=False, trace=False, tmpdir=None,
        phases=3):
    n_nodes, in_ch = x.shape
    heads, head_dim = att_src.shape
    out_ch = heads * head_dim
    shard = n_nodes // n_cores

    src = np.asarray(edge_index[0], np.int64)
    dst = np.asarray(edge_index[1], np.int64)
    sched, per_core = build_schedule(src, dst, n_nodes, n_cores, tiles_per_sc)
    nc = build_program(sched, heads, head_dim, msg_dbf16=msg_dbf16, phases=phases)

    x = np.asarray(x, np.float32)
    in_maps = []
    for c in range(n_cores):
        in_maps.append({
            "x": np.roll(x, -c * shard, axis=0),
            "w": np.asarray(W, np.float32),
            "att_src": np.asarray(att_src, np.float32),
            "att_dst": np.asarray(att_dst, np.float32),
            "gamma": np.asarray(gamma, np.float32).reshape(1, out_ch),
            "beta": (np.asarray(beta, np.float32)
                     + np.asarray(bias, np.float32) * 0.0).reshape(1, out_ch),
            "g_idx": per_core[c]["g_idx"],
            "g_dstloc": per_core[c]["g_dstloc"],
            "ident": np.eye(P, dtype=np.float32),
            "iota": np.tile(np.arange(sched["sc_width"], dtype=np.float32), (P, 1)),
        })
    res = run_bass_kernel_spmd(nc, in_maps, core_ids=list(range(n_cores)),
                               trace=trace, tmpdir=tmpdir)
    out = np.concatenate([res.results[c]["out"] for c in range(n_cores)], axis=0)
    return out, res




def _install_ntff_hook():
    """Best-effort NTFF profile hook for trace mode (missing in this image)."""
    import sys, types
    try:
        import antenv.axon_hooks  # noqa: F401
        return
    except ImportError:
        pass
    try:
        from trn_agent_boot.trn_boot import _ntff_profile_via_ctypes
        hook = _ntff_profile_via_ctypes('/opt/axon/libaxon_pjrt.so')
    except Exception:
        hook = None
    mod = types.ModuleType("antenv.axon_hooks")
    mod.get_axon_ntff_profile_hook = lambda: hook
    mod.set_axon_ntff_profile_hook = lambda h: None
    sys.modules["antenv.axon_hooks"] = mod

_PROGRAM_CACHE = {}


def kernel(**inputs):
    x = np.asarray(inputs["x"], np.float32)
    edge_index = np.asarray(inputs["edge_index"])
    W = np.asarray(inputs["W"], np.float32)
    att_src = np.asarray(inputs["att_src"], np.float32)
    att_dst = np.asarray(inputs["att_dst"], np.float32)
    bias = np.asarray(inputs["bias"], np.float32)
    gamma = np.asarray(inputs["gamma"], np.float32)
    beta = np.asarray(inputs["beta"], np.float32)
    import os
    trace = bool(os.environ.get("GAT_TRACE"))
    if trace:
        _install_ntff_hook()
    tmpdir = os.environ.get("GAT_TRACE_DIR")
    out, res = run(x, edge_index, W, att_src, att_dst, bias, gamma, beta,
                   trace=trace, tmpdir=tmpdir)
    kernel.last_result = res
    return out



# revision 8
# speedup vs baseline: 2.1951x; 2.1951x over previous
"""GATConvBlock (GAT attention + BatchNorm + LeakyReLU) on 8 Trainium2
NeuronCores. Self-contained: host-side edge scheduling + Bass/Tile program +
SPMD execution via concourse.

Strategy: dst-sharded graph parallelism. Each core owns 12500 dst nodes; x is
rotated per core so its shard is local rows 0:12500. Phase 1 builds DRAM
range-split tables with 512B rows [h(bf16)|a_src(f32)] using bf16 matmuls and
batched contiguous DMAs. Phase 2 gathers per-edge rows with int16 dma_gather
(range tables let gathers overlap phase 1), does segment softmax + aggregation
via one-hot matmuls in PSUM with unit-batched edge math. BatchNorm stats
AllReduce across cores; normalize + LeakyReLU from SBUF-resident out tiles.
"""
import concourse.tile as _tile_mod

import numpy as np
import ml_dtypes

import concourse.bass as bass
import concourse.bacc as bacc
import concourse.tile as tile
from concourse import mybir
from concourse.bass_utils import run_bass_kernel_spmd

# ---- walrus compat: split multi-wait sync_info (this toolchain rejects >1) ----
from concourse import mybir as _mb
from concourse.tile import TileContext as _TC
from concourse.vector_clock import ScopedClock as _SC

_MAX_WAITS = 1


def _patched_drain_and_barrier(self, tick_clock, wait_clock):
    drain_inst = self.nc.sync.drain()
    wait_clock.add_sem_waits(drain_inst.ins, _SC({None: tick_clock.global_clock}))
    si = drain_inst.ins.sync_info
    waits = list(si.on_wait or [])
    if len(waits) > _MAX_WAITS:
        si.on_wait = waits[:_MAX_WAITS]
        for w in waits[_MAX_WAITS:]:
            n = self.nc.sync.nop()
            n.ins.sync_info = _mb.SyncInfo(on_wait=[w], on_update=[])
        self.nc.sync.drain()
    self.nc.all_engine_barrier()
    popped = self.nc._tile_sem_poison_stack.pop()
    assert popped is self._sem_poison
    self.nc.clear_and_free_semaphores(list(self.sems.allocated().values()))
    self.nc.all_engine_barrier()


_orig_commit = _TC._commit_instruction


def _patched_commit(self, inst, lazy_reg_writes=True):
    si = inst.sync_info
    if si is not None and si.on_wait and len(si.on_wait) > _MAX_WAITS and \
            inst.engine != _mb.EngineType.Unassigned:
        waits = list(si.on_wait)
        si.on_wait = waits[:_MAX_WAITS]
        eng = self.nc.engines[inst.engine]
        for w in waits[_MAX_WAITS:]:
            n = eng.nop()
            n.ins.sync_info = _mb.SyncInfo(on_wait=[w], on_update=[])
    return _orig_commit(self, inst, lazy_reg_writes)


if getattr(_TC, "_gat_patched", False) is False:
    _TC._drain_and_barrier = _patched_drain_and_barrier
    _TC._commit_instruction = _patched_commit
    _TC._gat_patched = True
# ---- end walrus compat ----


f32 = mybir.dt.float32
f32r = mybir.dt.float32r
bf16 = mybir.dt.bfloat16
i16 = mybir.dt.int16
AF = mybir.ActivationFunctionType
ALU = mybir.AluOpType

P = 128
RANGE = 32768
NEG = 0.2
BN_EPS = 1e-5
ROW = 128           # table row width in f32 elems (512B)
HCOLS = 64          # h bf16 occupies f32 cols [0, 64)
ACOL = 64           # a_src f32 at cols [64, 68)
GATHER_PIECE = 896


def build_schedule(src_g, dst_g, n_nodes, n_cores, tiles_per_sc=4,
                   gather_piece=GATHER_PIECE):
    """Static edge schedule. 128-edge chunks of dst-sorted edges per
    (supercolumn, src-range) unit; each (chunk, target-tile) pair gets one dl
    column with tile-local dst offsets (-1000 for inactive/pad slots).
    Structure (caps/pairs) is the max/union across cores: one SPMD program."""
    shard = n_nodes // n_cores
    sc_width = tiles_per_sc * P
    n_sc = (shard + sc_width - 1) // sc_width
    n_ranges = (n_nodes + RANGE - 1) // RANGE
    n_tiles = (shard + P - 1) // P

    order = np.argsort(dst_g, kind="stable")
    src_s = src_g[order]
    dst_s = dst_g[order]
    core_bounds = np.searchsorted(dst_s, np.arange(n_cores + 1) * shard)

    counts = np.zeros((n_cores, n_sc, n_ranges), np.int64)
    segs = [[[None] * n_ranges for _ in range(n_sc)] for _ in range(n_cores)]
    for c in range(n_cores):
        lo, hi = core_bounds[c], core_bounds[c + 1]
        s = (src_s[lo:hi] - c * shard) % n_nodes   # rotated source index
        d = dst_s[lo:hi] - c * shard               # local dst 0..shard
        sc_id = d // sc_width
        r_id = s // RANGE
        o2 = np.lexsort((r_id, sc_id))             # stable: keeps dst order
        s, d, sc_id, r_id = s[o2], d[o2], sc_id[o2], r_id[o2]
        key = sc_id * n_ranges + r_id
        cnt = np.bincount(key, minlength=n_sc * n_ranges).reshape(n_sc, n_ranges)
        counts[c] = cnt
        offs = np.concatenate([[0], np.cumsum(cnt.ravel())])
        for sc in range(n_sc):
            for r in range(n_ranges):
                k = sc * n_ranges + r
                sl = slice(offs[k], offs[k + 1])
                segs[c][sc][r] = (s[sl], d[sl])

    caps = counts.max(axis=0)
    caps = ((caps + P - 1) // P) * P

    # pairs[(sc, r, k)] = sorted list of target tiles (union over cores)
    pairs = {}
    for sc in range(n_sc):
        t_lo = sc * tiles_per_sc
        for r in range(n_ranges):
            for k in range(int(caps[sc, r]) // P):
                tg = set()
                for c in range(n_cores):
                    dseg = segs[c][sc][r][1]
                    part = dseg[k * P:(k + 1) * P]
                    if len(part):
                        tg |= set(np.unique(part // P).tolist())
                if not tg:
                    tg = {min(t_lo, n_tiles - 1)}
                pairs[(sc, r, k)] = sorted(tg)

    # make sure every tile appears in >= 1 pair (psum start/stop defined)
    covered = set()
    for v in pairs.values():
        covered.update(v)
    for t in range(n_tiles):
        if t not in covered:
            sc = t // tiles_per_sc
            for r in range(n_ranges):
                if caps[sc, r] > 0:
                    k0 = (sc, r, 0)
                    pairs[k0] = sorted(set(pairs[k0]) | {t})
                    break

    # linearize pairs in emission order; record dl column index per pair
    pair_list = []         # (sc, r, k, t, col)
    col = 0
    for sc in range(n_sc):
        for r in range(n_ranges):
            for k in range(int(caps[sc, r]) // P):
                for t in pairs[(sc, r, k)]:
                    pair_list.append((sc, r, k, t, col))
                    col += 1
    n_pairs = col

    # idx packing offsets (16-wrapped int16 layout per unit)
    o16 = np.zeros((n_sc, n_ranges), np.int64)
    acc16 = 0
    for sc in range(n_sc):
        for r in range(n_ranges):
            o16[sc, r] = acc16
            acc16 += int(caps[sc, r]) // 16
    idxcols = acc16

    per_core = []
    for c in range(n_cores):
        idx_arr = np.zeros((16, idxcols), np.int16)
        dl_arr = np.full((P, n_pairs), -1000.0, np.float32)
        for sc in range(n_sc):
            for r in range(n_ranges):
                cap = int(caps[sc, r])
                if cap == 0:
                    continue
                sseg, _ = segs[c][sc][r]
                n = len(sseg)
                iv = np.zeros(cap, np.int16)
                iv[:n] = (sseg - r * RANGE).astype(np.int16)
                idx_arr[:, int(o16[sc, r]): int(o16[sc, r]) + cap // 16] = \
                    iv.reshape(cap // 16, 16).T
        for (sc, r, k, t, ci) in pair_list:
            _, dseg = segs[c][sc][r]
            n = len(dseg)
            dv = np.full(P, -100000.0, np.float32)
            lo = k * P
            hi = min(lo + P, n)
            if hi > lo:
                dv[:hi - lo] = dseg[lo:hi].astype(np.float32)
            dl = dv - t * P
            dl[(dl < 0) | (dl >= P)] = -1000.0
            dl_arr[:, ci] = dl
        per_core.append({
            "g_idx": np.tile(idx_arr, (8, 1)),
            "g_dl": dl_arr,
        })

    return {
        "n_sc": n_sc, "n_ranges": n_ranges, "caps": caps, "o16": o16,
        "idxcols": idxcols, "pairs": pairs, "pair_list": pair_list,
        "n_pairs": n_pairs, "shard": shard, "n_tiles": n_tiles,
        "tiles_per_sc": tiles_per_sc, "sc_width": sc_width,
        "n_nodes": n_nodes, "gather_piece": gather_piece,
    }, per_core


def build_program(sched, heads=4, head_dim=32):
    n_nodes = sched["n_nodes"]
    in_ch = P
    out_ch = heads * head_dim
    shard = sched["shard"]
    n_sc, n_ranges = sched["n_sc"], sched["n_ranges"]
    caps, o16 = sched["caps"], sched["o16"]
    pairs = sched["pairs"]
    n_tiles = sched["n_tiles"]
    tiles_per_sc = sched["tiles_per_sc"]
    gather_piece = sched["gather_piece"]
    H2 = 2 * heads
    NAGG = out_ch + heads        # 132 psum cols: [h-sum | exp-sum]

    colof = {}
    for (sc, r, k, t, ci) in sched["pair_list"]:
        colof[(sc, r, k, t)] = ci

    tile_pairs = {t: [] for t in range(n_tiles)}
    for sc in range(n_sc):
        for r in range(n_ranges):
            for k in range(int(caps[sc, r]) // P):
                for t in pairs[(sc, r, k)]:
                    tile_pairs[t].append((sc, r, k))

    nc = bacc.Bacc("TRN2", target_bir_lowering=False, num_devices=8,
                   num_swdge_queues=4)
    x_in = nc.dram_tensor("x", [n_nodes, in_ch], f32, kind="ExternalInput")
    w_in = nc.dram_tensor("w", [in_ch, out_ch], f32, kind="ExternalInput")
    asrc_in = nc.dram_tensor("att_src", [heads, head_dim], f32, kind="ExternalInput")
    adst_in = nc.dram_tensor("att_dst", [heads, head_dim], f32, kind="ExternalInput")
    gamma_in = nc.dram_tensor("gamma", [1, out_ch], f32, kind="ExternalInput")
    beta_in = nc.dram_tensor("beta", [1, out_ch], f32, kind="ExternalInput")
    gidx_in = nc.dram_tensor("g_idx", [P, sched["idxcols"]], i16, kind="ExternalInput")
    gdl_in = nc.dram_tensor("g_dl", [P, sched["n_pairs"]], f32, kind="ExternalInput")
    ident_in = nc.dram_tensor("ident", [P, P], f32, kind="ExternalInput")
    iota_in = nc.dram_tensor("iota_b", [P, P], bf16, kind="ExternalInput")
    out_fin = nc.dram_tensor("out", [shard, out_ch], f32, kind="ExternalOutput")

    range_rows = [min(RANGE, n_nodes - r * RANGE) for r in range(n_ranges)]

    with tile.TileContext(nc) as tc:
        with tc.tile_pool(name="dram", bufs=1, space="DRAM") as drp, \
             tc.tile_pool(name="const", bufs=1) as cs:
            tabs = [drp.tile([range_rows[r], ROW], f32, name=f"tab{r}")
                    for r in range(n_ranges)]
            bn_in = drp.tile([1, 2 * out_ch], f32)
            bn_out = drp.tile([1, 2 * out_ch], f32)

            # ---------------- constants ----------------
            identf = cs.tile([P, P], f32)
            nc.sync.dma_start(out=identf[:], in_=ident_in[:])
            identb = cs.tile([P, P], bf16)
            nc.vector.tensor_copy(out=identb[:], in_=identf[:])
            iota_b = cs.tile([P, P], bf16)
            nc.sync.dma_start(out=iota_b[:], in_=iota_in[:])
            alpha_c = cs.tile([P, 1], f32)
            nc.vector.memset(alpha_c[:], NEG)
            ones_bf = cs.tile([P, 1], bf16)
            nc.vector.memset(ones_bf[:], 1.0)
            ones1f = cs.tile([1, P], f32)
            nc.vector.memset(ones1f[:], 1.0)
            ones1r = cs.tile([1, P], f32r)
            nc.vector.tensor_copy(out=ones1r[:], in_=ones1f[:])
            eps_c = cs.tile([1, 1], f32)
            nc.vector.memset(eps_c[:], BN_EPS)

            gidx_sb = cs.tile([P, sched["idxcols"]], i16)
            nc.sync.dma_start(out=gidx_sb[:], in_=gidx_in[:])
            gdl_sb = cs.tile([P, sched["n_pairs"]], f32)
            nc.sync.dma_start(out=gdl_sb[:], in_=gdl_in[:])

            # a_dst per local dst node, bf16 hi/lo, SBUF-resident
            a_sb = cs.tile([P, n_tiles, H2], bf16)
            # output tiles, SBUF-resident f32
            o_sto = cs.tile([P, n_tiles, out_ch], f32)
            stats_acc = cs.tile([1, 2 * out_ch], f32)
            nc.vector.memset(stats_acc[:], 0.0)

            # ---------------- phase 0: weights ----------------
            with tc.tile_pool(name="ph0", bufs=1) as p0, \
                 tc.tile_pool(name="ph0ps", bufs=2, space="PSUM") as psc:
                A_sb = p0.tile([P, H2], f32)
                nc.vector.memset(A_sb[:], 0.0)
                for hd in range(heads):
                    nc.sync.dma_start(
                        out=A_sb[hd * head_dim:(hd + 1) * head_dim, hd:hd + 1],
                        in_=asrc_in[hd:hd + 1, :].rearrange("a b -> b a"))
                    nc.sync.dma_start(
                        out=A_sb[hd * head_dim:(hd + 1) * head_dim,
                                 heads + hd:heads + hd + 1],
                        in_=adst_in[hd:hd + 1, :].rearrange("a b -> b a"))
                Ab = p0.tile([P, H2], bf16)
                nc.vector.tensor_copy(out=Ab[:], in_=A_sb[:])
                Wf = p0.tile([P, P], f32)
                nc.sync.dma_start(out=Wf[:], in_=w_in[:])
                Wb = p0.tile([P, P], bf16)
                nc.vector.tensor_copy(out=Wb[:], in_=Wf[:])
                WT_ps = psc.tile([P, P], bf16)
                nc.tensor.transpose(out=WT_ps[:], in_=Wb[:], identity=identb[:])
                WT = p0.tile([P, P], bf16)
                nc.scalar.copy(out=WT[:], in_=WT_ps[:])
                WA_ps = psc.tile([P, H2], f32)
                nc.tensor.matmul(out=WA_ps[:], lhsT=WT[:], rhs=Ab[:],
                                 start=True, stop=True)
                # rhs_big bf16 [P, 136] = [W | W@A]
                rhs_big = cs.tile([P, P + H2], bf16)
                nc.vector.tensor_copy(out=rhs_big[:, 0:P], in_=Wb[:])
                nc.vector.tensor_copy(out=rhs_big[:, P:P + H2], in_=WA_ps[:])

            # ------- phase 1: table (range-major, contiguous batched DMA) ----
            XB = 4  # tiles per batch
            n_nt = (n_nodes + P - 1) // P
            n_full_batches = (n_nodes // (XB * P))
            with tc.tile_pool(name="p1x", bufs=3) as p1x, \
                 tc.tile_pool(name="p1w", bufs=6) as p1w, \
                 tc.tile_pool(name="p1s", bufs=3) as p1s, \
                 tc.tile_pool(name="p1ps", bufs=2, space="PSUM") as p1ps, \
                 tc.tile_pool(name="p1ps2", bufs=4, space="PSUM") as p1ps2:

                def do_tile(xb_slice, pk, j, nt, pt):
                    """xb_slice: [pt<=P, P] bf16 nodes-of-tile; writes pk[:,j,:]
                    and stashes a_dst hi/lo for local tiles."""
                    xT_ps = p1ps.tile([P, P], bf16, tag="xtp")
                    nc.tensor.transpose(out=xT_ps[:, :pt], in_=xb_slice,
                                        identity=identb[:pt, :pt])
                    xT = p1w.tile([P, P], bf16, tag="xt")
                    nc.scalar.copy(out=xT[:, :pt], in_=xT_ps[:, :pt])
                    h_ps = p1ps2.tile([P, P + H2], f32, tag="hp")
                    nc.tensor.matmul(out=h_ps[:pt], lhsT=xT[:, :pt],
                                     rhs=rhs_big[:], start=True, stop=True)
                    nc.vector.tensor_copy(
                        out=pk[:pt, j, 0:HCOLS].bitcast(bf16),
                        in_=h_ps[:pt, 0:P])
                    nc.scalar.copy(
                        out=pk[:pt, j, ACOL:ACOL + heads],
                        in_=h_ps[:pt, P:P + heads])
                    if nt is not None and nt < n_tiles:
                        # a_dst -> bf16 hi + lo correction
                        nc.vector.tensor_copy(out=a_sb[:pt, nt, 0:heads],
                                              in_=h_ps[:pt, P + heads:P + H2])
                        hi_f = p1w.tile([P, heads], f32, tag="hif")
                        nc.vector.tensor_copy(out=hi_f[:pt],
                                              in_=a_sb[:pt, nt, 0:heads])
                        lo_f = p1w.tile([P, heads], f32, tag="lof")
                        nc.vector.tensor_tensor(out=lo_f[:pt],
                                                in0=h_ps[:pt, P + heads:P + H2],
                                                in1=hi_f[:pt],
                                                op=ALU.subtract)
                        nc.vector.tensor_copy(out=a_sb[:pt, nt, heads:H2],
                                              in_=lo_f[:pt])

                # 4-tile batches, tile-major layout: xb[:, j, :] = tile nb*4+j
                for nb in range(n_full_batches):
                    r0 = nb * XB * P
                    xr = p1x.tile([P, XB, P], f32, tag="xr")
                    nc.sync.dma_start(
                        out=xr[:],
                        in_=x_in[r0:r0 + XB * P, :]
                        .rearrange("(k p) c -> p k c", p=P))
                    xb = p1w.tile([P, XB, P], bf16, tag="xb")
                    nc.scalar.copy(out=xb[:].rearrange("p k c -> p (k c)"),
                                   in_=xr[:].rearrange("p k c -> p (k c)"))
                    pk = p1s.tile([P, XB, ROW], f32, tag="pk")
                    for j in range(XB):
                        do_tile(xb[:, j, :], pk, j, nb * XB + j, P)
                    r_id = r0 // RANGE
                    base = r_id * RANGE
                    nc.scalar.dma_start(
                        out=tabs[r_id][r0 - base:r0 - base + XB * P, :]
                        .rearrange("(k p) c -> p k c", p=P),
                        in_=pk[:])

                # tail tiles (per-tile)
                for nt in range(n_full_batches * XB, n_nt):
                    r0 = nt * P
                    r1 = min(r0 + P, n_nodes)
                    pt = r1 - r0
                    xr = p1x.tile([P, P], f32, tag="xr1")
                    nc.sync.dma_start(out=xr[:pt], in_=x_in[r0:r1, :])
                    xb = p1w.tile([P, P], bf16, tag="xb1")
                    nc.scalar.copy(out=xb[:pt], in_=xr[:pt])
                    pk = p1s.tile([P, 1, ROW], f32, tag="pk1")
                    do_tile(xb[:pt], pk, 0, nt, pt)
                    r_id = r0 // RANGE
                    base = r_id * RANGE
                    nc.scalar.dma_start(out=tabs[r_id][r0 - base:r1 - base, :],
                                        in_=pk[:pt, 0, :])

            # ---------------- phase 2: edges ----------------
            gq = [0]
            with tc.tile_pool(name="gb", bufs=3) as gbp, \
                 tc.tile_pool(name="mcp", bufs=48) as mcp, \
                 tc.tile_pool(name="mctp", bufs=8) as mctp, \
                 tc.tile_pool(name="mep", bufs=2) as mep, \
                 tc.tile_pool(name="sp", bufs=4) as spl, \
                 tc.tile_pool(name="ep", bufs=6) as epl, \
                 tc.tile_pool(name="aggps", bufs=tiles_per_sc, space="PSUM") as aggp, \
                 tc.tile_pool(name="trps", bufs=2, space="PSUM") as trp, \
                 tc.tile_pool(name="aeps", bufs=1, space="PSUM") as aep, \
                 tc.tile_pool(name="smps", bufs=1, space="PSUM") as smp:
                psums = {}
                done = {t: 0 for t in range(n_tiles)}
                for sc in range(n_sc):
                    t_lo = sc * tiles_per_sc
                    t_hi = min(t_lo + tiles_per_sc, n_tiles)
                    for t in range(t_lo, t_hi):
                        psums[t] = aggp.tile([P, NAGG], f32, tag="agg",
                                             name=f"agg{t}")
                    for r in range(n_ranges):
                        cap = int(caps[sc, r])
                        if cap == 0:
                            continue
                        K = cap // P
                        g = gbp.tile([P, K, ROW], f32, tag="g",
                                     name=f"g_{sc}_{r}")
                        off = 0
                        while off < cap:
                            piece = min(gather_piece, cap - off)
                            nc.gpsimd.dma_gather(
                                out_ap=g[:, off // P: (off + piece + P - 1) // P, :],
                                in_ap=tabs[r][:],
                                idxs_ap=gidx_sb[:, int(o16[sc, r]) + off // 16:
                                                int(o16[sc, r]) + (off + piece) // 16],
                                num_idxs=piece,
                                num_idxs_reg=piece,
                                elem_size=ROW,
                                queue_num=gq[0] % 4,
                            )
                            gq[0] += 1
                            off += piece

                        unit_pairs = [(k, t) for k in range(K)
                                      for t in pairs[(sc, r, k)]]
                        npair = len(unit_pairs)
                        ae_ps = aep.tile([P, K * H2], f32, tag="ae",
                                         name=f"ae_{sc}_{r}")
                        mc_t = {}
                        kfirst = {}
                        klast = {}
                        for (k, t) in unit_pairs:
                            kfirst.setdefault(k, (k, t))
                            klast[k] = (k, t)
                        for pb in range(0, npair, 4):
                            grpp = unit_pairs[pb:pb + 4]
                            tr_ps = trp.tile([P, 4 * P], bf16, tag="tr")
                            for i, (k, t) in enumerate(grpp):
                                ci = colof[(sc, r, k, t)]
                                mc = mcp.tile([P, P], bf16, tag="mc",
                                              name=f"mc_{sc}_{r}_{k}_{t}")
                                nc.vector.tensor_scalar(
                                    out=mc[:], in0=iota_b[:],
                                    scalar1=gdl_sb[:, ci:ci + 1],
                                    scalar2=None, op0=ALU.is_equal)
                                mc_t[(k, t)] = mc
                                nc.tensor.transpose(
                                    out=tr_ps[:, i * P:(i + 1) * P],
                                    in_=mc[:], identity=identb[:])
                            mct = mctp.tile([P, 4 * P], bf16, tag="mct")
                            nc.scalar.copy(out=mct[:, 0:len(grpp) * P],
                                           in_=tr_ps[:, 0:len(grpp) * P])
                            for i, (k, t) in enumerate(grpp):
                                nc.tensor.matmul(
                                    out=ae_ps[:, k * H2:(k + 1) * H2],
                                    lhsT=mct[:, i * P:(i + 1) * P],
                                    rhs=a_sb[:, t, :],
                                    start=kfirst[k] == (k, t),
                                    stop=klast[k] == (k, t))

                        # --- batched edge math over the unit ---
                        aesl = ae_ps[:]
                        s_t = spl.tile([P, K * heads], f32, tag="s")
                        nc.vector.tensor_tensor(
                            out=s_t[:].rearrange("p (k h) -> p k h", k=K),
                            in0=g[:, :, ACOL:ACOL + heads],
                            in1=bass.AP(tensor=aesl.tensor,
                                        offset=aesl.offset,
                                        ap=[aesl.ap[0], [H2, K], [1, heads]]),
                            op=ALU.add)
                        nc.vector.tensor_tensor(
                            out=s_t[:].rearrange("p (k h) -> p k h", k=K),
                            in0=s_t[:].rearrange("p (k h) -> p k h", k=K),
                            in1=bass.AP(tensor=aesl.tensor,
                                        offset=aesl.offset + heads,
                                        ap=[aesl.ap[0], [H2, K], [1, heads]]),
                            op=ALU.add)
                        lr = spl.tile([P, K * heads], f32, tag="lr")
                        nc.scalar.activation(out=lr[:], in_=s_t[:],
                                             func=AF.Prelu,
                                             alpha=alpha_c[:, 0:1])
                        me = mep.tile([P, K, NAGG], bf16, tag="me",
                                      name=f"me_{sc}_{r}")
                        nc.scalar.activation(
                            out=me[:, :, out_ch:NAGG],
                            in_=lr[:].rearrange("p (k h) -> p k h", k=K),
                            func=AF.Exp)
                        g2 = g[:, 0, 0:HCOLS].bitcast(bf16)   # [P, 128] bf16
                        for hd in range(heads):
                            exv = me[:, :, out_ch + hd:out_ch + hd + 1]
                            nc.vector.tensor_tensor(
                                out=me[:, :, hd * head_dim:(hd + 1) * head_dim],
                                in0=bass.AP(
                                    tensor=g2.tensor,
                                    offset=g2.offset + hd * head_dim,
                                    ap=[g2.ap[0], [2 * ROW, K], [1, head_dim]]),
                                in1=bass.AP(
                                    tensor=exv.tensor, offset=exv.offset,
                                    ap=[exv.ap[0], exv.ap[1], [0, head_dim]]),
                                op=ALU.mult)

                        # --- aggregation matmuls ---
                        for (k, t) in unit_pairs:
                            first = done[t] == 0
                            last = done[t] == len(tile_pairs[t]) - 1
                            nc.tensor.matmul(out=psums[t][:],
                                             lhsT=mc_t[(k, t)][:],
                                             rhs=me[:, k, :],
                                             start=first, stop=last)
                            done[t] += 1

                    # ---- per-tile epilogue (after all ranges of sc) ----
                    for t in range(t_lo, t_hi):
                        d0 = t * P
                        pt = min(d0 + P, shard) - d0
                        ps = psums[t]
                        den = epl.tile([P, heads], f32, tag="den")
                        nc.vector.tensor_scalar(
                            out=den[:pt], in0=ps[:pt, out_ch:NAGG],
                            scalar1=1e-16, scalar2=None, op0=ALU.add)
                        rden = epl.tile([P, heads], f32, tag="rden")
                        nc.vector.reciprocal(out=rden[:pt], in_=den[:pt])
                        rv = rden[:pt]
                        rd_b = bass.AP(tensor=rv.tensor, offset=rv.offset,
                                       ap=[rv.ap[0], rv.ap[1], [0, head_dim]])
                        nc.vector.tensor_tensor(
                            out=o_sto[:pt, t, :].rearrange(
                                "p (h c) -> p h c", h=heads),
                            in0=ps[:pt, 0:out_ch].rearrange(
                                "p (h c) -> p h c", h=heads),
                            in1=rd_b, op=ALU.mult)
                        st_in = epl.tile([P, 2 * out_ch], bf16, tag="stin")
                        if pt < P:
                            nc.vector.memset(st_in[:], 0.0)
                        nc.scalar.copy(out=st_in[:pt, 0:out_ch],
                                       in_=o_sto[:pt, t, :])
                        nc.vector.tensor_tensor(out=st_in[:pt, out_ch:],
                                                in0=o_sto[:pt, t, :],
                                                in1=o_sto[:pt, t, :],
                                                op=ALU.mult)
                        sm_ps = smp.tile([1, 2 * out_ch], f32, tag="sm")
                        nc.tensor.matmul(out=sm_ps[:], lhsT=ones_bf[:],
                                         rhs=st_in[:], start=True, stop=True)
                        nc.vector.tensor_tensor(out=stats_acc[:],
                                                in0=stats_acc[:], in1=sm_ps[:],
                                                op=ALU.add)

            # ---------------- phase 2.5: BN stats allreduce ----------------
            with tc.tile_pool(name="bn", bufs=1) as bnp, \
                 tc.tile_pool(name="bnps", bufs=1, space="PSUM") as bnps:
                nc.sync.dma_start(out=bn_in[:], in_=stats_acc[:])
                nc.gpsimd.collective_compute(
                    "AllReduce", ALU.add,
                    replica_groups=[list(range(8))],
                    ins=[bn_in[:]], outs=[bn_out[:]],
                )
                tots = bnp.tile([1, 2 * out_ch], f32)
                nc.sync.dma_start(out=tots[:], in_=bn_out[:])
                gam = bnp.tile([1, out_ch], f32)
                nc.sync.dma_start(out=gam[:], in_=gamma_in[:])
                bet = bnp.tile([1, out_ch], f32)
                nc.sync.dma_start(out=bet[:], in_=beta_in[:])
                mu = bnp.tile([1, out_ch], f32)
                nc.vector.tensor_scalar(out=mu[:], in0=tots[:, 0:out_ch],
                                        scalar1=1.0 / n_nodes, scalar2=None,
                                        op0=ALU.mult)
                ex2 = bnp.tile([1, out_ch], f32)
                nc.vector.tensor_scalar(out=ex2[:], in0=tots[:, out_ch:],
                                        scalar1=1.0 / n_nodes, scalar2=None,
                                        op0=ALU.mult)
                musq = bnp.tile([1, out_ch], f32)
                nc.vector.tensor_tensor(out=musq[:], in0=mu[:], in1=mu[:],
                                        op=ALU.mult)
                var = bnp.tile([1, out_ch], f32)
                nc.vector.tensor_tensor(out=var[:], in0=ex2[:], in1=musq[:],
                                        op=ALU.subtract)
                sd = bnp.tile([1, out_ch], f32)
                nc.scalar.activation(out=sd[:], in_=var[:], func=AF.Sqrt,
                                     bias=eps_c[0:1, 0:1], scale=1.0)
                rs = bnp.tile([1, out_ch], f32)
                nc.vector.reciprocal(out=rs[:], in_=sd[:])
                scale_r = bnp.tile([1, out_ch], f32)
                nc.vector.tensor_tensor(out=scale_r[:], in0=rs[:], in1=gam[:],
                                        op=ALU.mult)
                mshift = bnp.tile([1, out_ch], f32)
                nc.vector.tensor_tensor(out=mshift[:], in0=mu[:], in1=scale_r[:],
                                        op=ALU.mult)
                shift_r = bnp.tile([1, out_ch], f32)
                nc.vector.tensor_tensor(out=shift_r[:], in0=bet[:], in1=mshift[:],
                                        op=ALU.subtract)
                scsh = bnp.tile([1, 2 * out_ch], f32r)
                nc.vector.tensor_copy(out=scsh[:, 0:out_ch], in_=scale_r[:])
                nc.vector.tensor_copy(out=scsh[:, out_ch:], in_=shift_r[:])
                bc_ps = bnps.tile([P, 2 * out_ch], f32)
                nc.tensor.matmul(out=bc_ps[:], lhsT=ones1r[:], rhs=scsh[:],
                                 start=True, stop=True)
                scale_bc = cs.tile([P, out_ch], f32)
                nc.vector.tensor_copy(out=scale_bc[:], in_=bc_ps[:, 0:out_ch])
                shift_bc = cs.tile([P, out_ch], f32)
                nc.scalar.copy(out=shift_bc[:], in_=bc_ps[:, out_ch:])

            # ---------------- phase 3: normalize (from SBUF) ----------------
            with tc.tile_pool(name="p3", bufs=3) as p3:
                for sc in range(n_sc):
                    t_lo = sc * tiles_per_sc
                    t_hi = min(t_lo + tiles_per_sc, n_tiles)
                    nbt = t_hi - t_lo
                    y_t = p3.tile([P, tiles_per_sc, out_ch], f32, tag="y3")
                    for t in range(t_lo, t_hi):
                        j = t - t_lo
                        nc.vector.tensor_tensor(out=y_t[:, j, :],
                                                in0=o_sto[:, t, :],
                                                in1=scale_bc[:], op=ALU.mult)
                        nc.vector.tensor_tensor(out=y_t[:, j, :],
                                                in0=y_t[:, j, :],
                                                in1=shift_bc[:], op=ALU.add)
                        nc.scalar.activation(out=y_t[:, j, :], in_=y_t[:, j, :],
                                             func=AF.Prelu,
                                             alpha=alpha_c[:, 0:1])
                    d0 = t_lo * P
                    d1 = min(t_hi * P, shard)
                    nfull = (d1 - d0) // P
                    if nfull:
                        nc.sync.dma_start(
                            out=out_fin[d0:d0 + nfull * P, :]
                            .rearrange("(k p) c -> p k c", p=P),
                            in_=y_t[:, 0:nfull, :])
                    rem = (d1 - d0) - nfull * P
                    if rem:
                        nc.sync.dma_start(
                            out=out_fin[d0 + nfull * P:d1, :],
                            in_=y_t[:rem, nfull, :])

    nc.compile()
    return nc


def run(x, edge_index, W, att_src, att_dst, bias, gamma, beta,
        n_cores=8, tiles_per_sc=4, trace=False, tmpdir=None):
    n_nodes, in_ch = x.shape
    heads, head_dim = att_src.shape
    out_ch = heads * head_dim
    shard = n_nodes // n_cores

    src = np.asarray(edge_index[0], np.int64)
    dst = np.asarray(edge_index[1], np.int64)
    sched, per_core = build_schedule(src, dst, n_nodes, n_cores, tiles_per_sc)
    nc = build_program(sched, heads, head_dim)

    x = np.asarray(x, np.float32)
    iota_b = np.tile(np.arange(P, dtype=np.float32), (P, 1)) \
        .astype(ml_dtypes.bfloat16)
    in_maps = []
    for c in range(n_cores):
        in_maps.append({
            "x": np.roll(x, -c * shard, axis=0),
            "w": np.asarray(W, np.float32),
            "att_src": np.asarray(att_src, np.float32),
            "att_dst": np.asarray(att_dst, np.float32),
            "gamma": np.asarray(gamma, np.float32).reshape(1, out_ch),
            "beta": (np.asarray(beta, np.float32)
                     + np.asarray(bias, np.float32) * 0.0).reshape(1, out_ch),
            "g_idx": per_core[c]["g_idx"],
            "g_dl": per_core[c]["g_dl"],
            "ident": np.eye(P, dtype=np.float32),
            "iota_b": iota_b,
        })
    res = run_bass_kernel_spmd(nc, in_maps, core_ids=list(range(n_cores)),
                               trace=trace, tmpdir=tmpdir)
    out = np.concatenate([res.results[c]["out"] for c in range(n_cores)], axis=0)
    return out, res


def _install_ntff_hook():
    """Best-effort NTFF profile hook for trace mode (missing in this image)."""
    import sys, types
    try:
        import antenv.axon_hooks  # noqa: F401
        return
    except ImportError:
        pass
    try:
        from trn_agent_boot.trn_boot import _ntff_profile_via_ctypes
        hook = _ntff_profile_via_ctypes('/opt/axon/libaxon_pjrt.so')
    except Exception:
        hook = None
    mod = types.ModuleType("antenv.axon_hooks")
    mod.get_axon_ntff_profile_hook = lambda: hook
    mod.set_axon_ntff_profile_hook = lambda h: None
    sys.modules["antenv.axon_hooks"] = mod


def kernel(**inputs):
    x = np.asarray(inputs["x"], np.float32)
    edge_index = np.asarray(inputs["edge_index"])
    W = np.asarray(inputs["W"], np.float32)
    att_src = np.asarray(inputs["att_src"], np.float32)
    att_dst = np.asarray(inputs["att_dst"], np.float32)
    bias = np.asarray(inputs["bias"], np.float32)
    gamma = np.asarray(inputs["gamma"], np.float32)
    beta = np.asarray(inputs["beta"], np.float32)
    import os
    trace = bool(os.environ.get("GAT_TRACE"))
    if trace:
        _install_ntff_hook()
    tmpdir = os.environ.get("GAT_TRACE_DIR")
    out, res = run(x, edge_index, W, att_src, att_dst, bias, gamma, beta,
                   trace=trace, tmpdir=tmpdir)
    kernel.last_result = res
    return out


# revision 11
# speedup vs baseline: 2.7323x; 1.2447x over previous
"""GATConvBlock (GAT attention + BatchNorm + LeakyReLU) on 8 Trainium2
NeuronCores. Self-contained: host-side edge scheduling + Bass/Tile program +
SPMD execution via concourse.

Strategy: dst-sharded graph parallelism. Each core owns 12500 dst nodes; x is
rotated per core so its shard is local rows 0:12500. Phase 1 builds DRAM
range-split tables with 512B rows [h(bf16)|a_src(f32)] using bf16 matmuls and
batched contiguous DMAs. Phase 2 gathers per-edge rows with int16 dma_gather
(range tables let gathers overlap phase 1), does segment softmax + aggregation
via one-hot matmuls in PSUM with unit-batched edge math. BatchNorm stats
AllReduce across cores; normalize + LeakyReLU from SBUF-resident out tiles.
"""
import concourse.tile as _tile_mod

import numpy as np
import ml_dtypes

import concourse.bass as bass
import concourse.bacc as bacc
import concourse.tile as tile
from concourse import mybir
from concourse.bass_utils import run_bass_kernel_spmd

# ---- walrus compat: split multi-wait sync_info (this toolchain rejects >1) ----
from concourse import mybir as _mb
from concourse.tile import TileContext as _TC
from concourse.vector_clock import ScopedClock as _SC

_MAX_WAITS = 1


def _patched_drain_and_barrier(self, tick_clock, wait_clock):
    drain_inst = self.nc.sync.drain()
    wait_clock.add_sem_waits(drain_inst.ins, _SC({None: tick_clock.global_clock}))
    si = drain_inst.ins.sync_info
    waits = list(si.on_wait or [])
    if len(waits) > _MAX_WAITS:
        si.on_wait = waits[:_MAX_WAITS]
        for w in waits[_MAX_WAITS:]:
            n = self.nc.sync.nop()
            n.ins.sync_info = _mb.SyncInfo(on_wait=[w], on_update=[])
        self.nc.sync.drain()
    self.nc.all_engine_barrier()
    popped = self.nc._tile_sem_poison_stack.pop()
    assert popped is self._sem_poison
    self.nc.clear_and_free_semaphores(list(self.sems.allocated().values()))
    self.nc.all_engine_barrier()


_orig_commit = _TC._commit_instruction


def _patched_commit(self, inst, lazy_reg_writes=True):
    si = inst.sync_info
    if si is not None and si.on_wait and len(si.on_wait) > _MAX_WAITS and \
            inst.engine != _mb.EngineType.Unassigned:
        waits = list(si.on_wait)
        si.on_wait = waits[:_MAX_WAITS]
        eng = self.nc.engines[inst.engine]
        for w in waits[_MAX_WAITS:]:
            n = eng.nop()
            n.ins.sync_info = _mb.SyncInfo(on_wait=[w], on_update=[])
    return _orig_commit(self, inst, lazy_reg_writes)


if getattr(_TC, "_gat_patched", False) is False:
    _TC._drain_and_barrier = _patched_drain_and_barrier
    _TC._commit_instruction = _patched_commit
    _TC._gat_patched = True
# ---- end walrus compat ----


f32 = mybir.dt.float32
f32r = mybir.dt.float32r
bf16 = mybir.dt.bfloat16
i16 = mybir.dt.int16
AF = mybir.ActivationFunctionType
ALU = mybir.AluOpType

P = 128
RANGE = 32768
NEG = 0.2
BN_EPS = 1e-5
ROW = 128           # table row width in f32 elems (512B)
HCOLS = 64          # h bf16 occupies f32 cols [0, 64)
ACOL = 64           # a_src f32 at cols [64, 68)
GATHER_PIECE = 896


def build_schedule(src_g, dst_g, n_nodes, n_cores, tiles_per_sc=4,
                   gather_piece=GATHER_PIECE):
    """Static edge schedule. 128-edge chunks of dst-sorted edges per
    (supercolumn, src-range) unit; each (chunk, target-tile) pair gets one dl
    column with tile-local dst offsets (-1000 for inactive/pad slots).
    Structure (caps/pairs) is the max/union across cores: one SPMD program."""
    shard = n_nodes // n_cores
    sc_width = tiles_per_sc * P
    n_sc = (shard + sc_width - 1) // sc_width
    n_ranges = (n_nodes + RANGE - 1) // RANGE
    n_tiles = (shard + P - 1) // P

    order = np.argsort(dst_g, kind="stable")
    src_s = src_g[order]
    dst_s = dst_g[order]
    core_bounds = np.searchsorted(dst_s, np.arange(n_cores + 1) * shard)

    counts = np.zeros((n_cores, n_sc, n_ranges), np.int64)
    segs = [[[None] * n_ranges for _ in range(n_sc)] for _ in range(n_cores)]
    for c in range(n_cores):
        lo, hi = core_bounds[c], core_bounds[c + 1]
        s = (src_s[lo:hi] - c * shard) % n_nodes   # rotated source index
        d = dst_s[lo:hi] - c * shard               # local dst 0..shard
        sc_id = d // sc_width
        r_id = s // RANGE
        o2 = np.lexsort((r_id, sc_id))             # stable: keeps dst order
        s, d, sc_id, r_id = s[o2], d[o2], sc_id[o2], r_id[o2]
        key = sc_id * n_ranges + r_id
        cnt = np.bincount(key, minlength=n_sc * n_ranges).reshape(n_sc, n_ranges)
        counts[c] = cnt
        offs = np.concatenate([[0], np.cumsum(cnt.ravel())])
        for sc in range(n_sc):
            for r in range(n_ranges):
                k = sc * n_ranges + r
                sl = slice(offs[k], offs[k + 1])
                segs[c][sc][r] = (s[sl], d[sl])

    caps = counts.max(axis=0)
    caps = ((caps + P - 1) // P) * P

    # pairs[(sc, r, k)] = sorted list of target tiles (union over cores)
    pairs = {}
    for sc in range(n_sc):
        t_lo = sc * tiles_per_sc
        for r in range(n_ranges):
            for k in range(int(caps[sc, r]) // P):
                tg = set()
                for c in range(n_cores):
                    dseg = segs[c][sc][r][1]
                    part = dseg[k * P:(k + 1) * P]
                    if len(part):
                        tg |= set(np.unique(part // P).tolist())
                if not tg:
                    tg = {min(t_lo, n_tiles - 1)}
                pairs[(sc, r, k)] = sorted(tg)

    # make sure every tile appears in >= 1 pair (psum start/stop defined)
    covered = set()
    for v in pairs.values():
        covered.update(v)
    for t in range(n_tiles):
        if t not in covered:
            sc = t // tiles_per_sc
            for r in range(n_ranges):
                if caps[sc, r] > 0:
                    k0 = (sc, r, 0)
                    pairs[k0] = sorted(set(pairs[k0]) | {t})
                    break

    # linearize pairs in emission order; record dl column index per pair
    pair_list = []         # (sc, r, k, t, col)
    col = 0
    for sc in range(n_sc):
        for r in range(n_ranges):
            for k in range(int(caps[sc, r]) // P):
                for t in pairs[(sc, r, k)]:
                    pair_list.append((sc, r, k, t, col))
                    col += 1
    n_pairs = col

    # idx packing offsets (16-wrapped int16 layout per unit)
    o16 = np.zeros((n_sc, n_ranges), np.int64)
    acc16 = 0
    for sc in range(n_sc):
        for r in range(n_ranges):
            o16[sc, r] = acc16
            acc16 += int(caps[sc, r]) // 16
    idxcols = acc16

    per_core = []
    for c in range(n_cores):
        idx_arr = np.zeros((16, idxcols), np.int16)
        dl_arr = np.full((P, n_pairs), -1000.0, np.float32)
        for sc in range(n_sc):
            for r in range(n_ranges):
                cap = int(caps[sc, r])
                if cap == 0:
                    continue
                sseg, _ = segs[c][sc][r]
                n = len(sseg)
                iv = np.zeros(cap, np.int16)
                iv[:n] = (sseg - r * RANGE).astype(np.int16)
                idx_arr[:, int(o16[sc, r]): int(o16[sc, r]) + cap // 16] = \
                    iv.reshape(cap // 16, 16).T
        for (sc, r, k, t, ci) in pair_list:
            _, dseg = segs[c][sc][r]
            n = len(dseg)
            dv = np.full(P, -100000.0, np.float32)
            lo = k * P
            hi = min(lo + P, n)
            if hi > lo:
                dv[:hi - lo] = dseg[lo:hi].astype(np.float32)
            dl = dv - t * P
            dl[(dl < 0) | (dl >= P)] = -1000.0
            dl_arr[:, ci] = dl
        per_core.append({
            "g_idx": np.tile(idx_arr, (8, 1)),
            "g_dl": dl_arr.astype(ml_dtypes.bfloat16),
        })

    return {
        "n_sc": n_sc, "n_ranges": n_ranges, "caps": caps, "o16": o16,
        "idxcols": idxcols, "pairs": pairs, "pair_list": pair_list,
        "n_pairs": n_pairs, "shard": shard, "n_tiles": n_tiles,
        "tiles_per_sc": tiles_per_sc, "sc_width": sc_width,
        "n_nodes": n_nodes, "gather_piece": gather_piece,
    }, per_core


def build_program(sched, heads=4, head_dim=32):
    n_nodes = sched["n_nodes"]
    in_ch = P
    out_ch = heads * head_dim
    shard = sched["shard"]
    n_sc, n_ranges = sched["n_sc"], sched["n_ranges"]
    caps, o16 = sched["caps"], sched["o16"]
    pairs = sched["pairs"]
    n_tiles = sched["n_tiles"]
    tiles_per_sc = sched["tiles_per_sc"]
    gather_piece = sched["gather_piece"]
    H2 = 2 * heads
    NAGG = out_ch + heads        # 132 psum cols: [h-sum | exp-sum]

    colof = {}
    for (sc, r, k, t, ci) in sched["pair_list"]:
        colof[(sc, r, k, t)] = ci

    tile_pairs = {t: [] for t in range(n_tiles)}
    for sc in range(n_sc):
        for r in range(n_ranges):
            for k in range(int(caps[sc, r]) // P):
                for t in pairs[(sc, r, k)]:
                    tile_pairs[t].append((sc, r, k))

    nc = bacc.Bacc("TRN2", target_bir_lowering=False, num_devices=8,
                   num_swdge_queues=4)
    x_in = nc.dram_tensor("x", [n_nodes, in_ch], f32, kind="ExternalInput")
    w_in = nc.dram_tensor("w", [in_ch, out_ch], f32, kind="ExternalInput")
    asrc_in = nc.dram_tensor("att_src", [heads, head_dim], f32, kind="ExternalInput")
    adst_in = nc.dram_tensor("att_dst", [heads, head_dim], f32, kind="ExternalInput")
    gamma_in = nc.dram_tensor("gamma", [1, out_ch], f32, kind="ExternalInput")
    beta_in = nc.dram_tensor("beta", [1, out_ch], f32, kind="ExternalInput")
    gidx_in = nc.dram_tensor("g_idx", [P, sched["idxcols"]], i16, kind="ExternalInput")
    gdl_in = nc.dram_tensor("g_dl", [P, sched["n_pairs"]], bf16, kind="ExternalInput")
    ident_in = nc.dram_tensor("ident", [P, P], f32, kind="ExternalInput")
    iota_in = nc.dram_tensor("iota_b", [P, P], bf16, kind="ExternalInput")
    out_fin = nc.dram_tensor("out", [shard, out_ch], f32, kind="ExternalOutput")

    range_rows = [min(RANGE, n_nodes - r * RANGE) for r in range(n_ranges)]

    with tile.TileContext(nc) as tc:
        with tc.tile_pool(name="dram", bufs=1, space="DRAM") as drp, \
             tc.tile_pool(name="const", bufs=1) as cs:
            tabs = [drp.tile([range_rows[r], ROW], f32, name=f"tab{r}")
                    for r in range(n_ranges)]
            bn_in = drp.tile([1, 2 * out_ch], f32)
            bn_out = drp.tile([1, 2 * out_ch], f32)

            # ---------------- constants ----------------
            identf = cs.tile([P, P], f32)
            nc.sync.dma_start(out=identf[:], in_=ident_in[:])
            identb = cs.tile([P, P], bf16)
            nc.vector.tensor_copy(out=identb[:], in_=identf[:])
            iota_b = cs.tile([P, P], bf16)
            nc.sync.dma_start(out=iota_b[:], in_=iota_in[:])
            alpha_c = cs.tile([P, 1], f32)
            nc.vector.memset(alpha_c[:], NEG)
            ones_bf = cs.tile([P, 1], bf16)
            nc.vector.memset(ones_bf[:], 1.0)
            ones1f = cs.tile([1, P], f32)
            nc.vector.memset(ones1f[:], 1.0)
            ones1r = cs.tile([1, P], f32r)
            nc.vector.tensor_copy(out=ones1r[:], in_=ones1f[:])
            eps_c = cs.tile([1, 1], f32)
            nc.vector.memset(eps_c[:], BN_EPS)

            gidx_sb = cs.tile([P, sched["idxcols"]], i16)
            nc.sync.dma_start(out=gidx_sb[:], in_=gidx_in[:])
            gdl_sb = cs.tile([P, sched["n_pairs"]], bf16)
            nc.sync.dma_start(out=gdl_sb[:], in_=gdl_in[:])

            # a_dst per local dst node, bf16 hi/lo, SBUF-resident
            a_sb = cs.tile([P, n_tiles, H2], bf16)
            # output tiles, SBUF-resident f32
            o_sto = cs.tile([P, n_tiles, out_ch], f32)
            stats_acc = cs.tile([1, 2 * out_ch], f32)
            nc.vector.memset(stats_acc[:], 0.0)

            # ---------------- phase 0: weights ----------------
            with tc.tile_pool(name="ph0", bufs=1) as p0, \
                 tc.tile_pool(name="ph0ps", bufs=2, space="PSUM") as psc:
                A_sb = p0.tile([P, H2], f32)
                nc.vector.memset(A_sb[:], 0.0)
                for hd in range(heads):
                    nc.sync.dma_start(
                        out=A_sb[hd * head_dim:(hd + 1) * head_dim, hd:hd + 1],
                        in_=asrc_in[hd:hd + 1, :].rearrange("a b -> b a"))
                    nc.sync.dma_start(
                        out=A_sb[hd * head_dim:(hd + 1) * head_dim,
                                 heads + hd:heads + hd + 1],
                        in_=adst_in[hd:hd + 1, :].rearrange("a b -> b a"))
                Ab = p0.tile([P, H2], bf16)
                nc.vector.tensor_copy(out=Ab[:], in_=A_sb[:])
                Wf = p0.tile([P, P], f32)
                nc.sync.dma_start(out=Wf[:], in_=w_in[:])
                Wb = p0.tile([P, P], bf16)
                nc.vector.tensor_copy(out=Wb[:], in_=Wf[:])
                WT_ps = psc.tile([P, P], bf16)
                nc.tensor.transpose(out=WT_ps[:], in_=Wb[:], identity=identb[:])
                WT = p0.tile([P, P], bf16)
                nc.scalar.copy(out=WT[:], in_=WT_ps[:])
                WA_ps = psc.tile([P, H2], f32)
                nc.tensor.matmul(out=WA_ps[:], lhsT=WT[:], rhs=Ab[:],
                                 start=True, stop=True)
                # rhs_big bf16 [P, 136] = [W | W@A]
                rhs_big = cs.tile([P, P + H2], bf16)
                nc.vector.tensor_copy(out=rhs_big[:, 0:P], in_=Wb[:])
                nc.vector.tensor_copy(out=rhs_big[:, P:P + H2], in_=WA_ps[:])

            from contextlib import ExitStack
            _es = ExitStack()
            gbp = _es.enter_context(tc.tile_pool(name="gb", bufs=3))
            mcp = _es.enter_context(tc.tile_pool(name="mcp", bufs=2))
            mctp = _es.enter_context(tc.tile_pool(name="mctp", bufs=8))
            mep = _es.enter_context(tc.tile_pool(name="mep", bufs=2))
            spl = _es.enter_context(tc.tile_pool(name="sp", bufs=4))
            epl = _es.enter_context(tc.tile_pool(name="ep", bufs=6))

            # ------- phase 1: table (range-major, contiguous batched DMA) ----
            XB = 4  # tiles per batch
            n_nt = (n_nodes + P - 1) // P
            n_full_batches = (n_nodes // (XB * P))
            with tc.tile_pool(name="p1x", bufs=3) as p1x, \
                 tc.tile_pool(name="p1w", bufs=6) as p1w, \
                 tc.tile_pool(name="p1s", bufs=3) as p1s, \
                 tc.tile_pool(name="p1ps", bufs=2, space="PSUM") as p1ps, \
                 tc.tile_pool(name="p1ps2", bufs=4, space="PSUM") as p1ps2:

                def do_tile(xb_slice, pk, j, nt, pt):
                    """xb_slice: [pt<=P, P] bf16 nodes-of-tile; writes pk[:,j,:]
                    and stashes a_dst hi/lo for local tiles."""
                    xT_ps = p1ps.tile([P, P], bf16, tag="xtp")
                    nc.tensor.transpose(out=xT_ps[:, :pt], in_=xb_slice,
                                        identity=identb[:pt, :pt])
                    xT = p1w.tile([P, P], bf16, tag="xt")
                    nc.scalar.copy(out=xT[:, :pt], in_=xT_ps[:, :pt])
                    h_ps = p1ps2.tile([P, P + H2], f32, tag="hp")
                    nc.tensor.matmul(out=h_ps[:pt], lhsT=xT[:, :pt],
                                     rhs=rhs_big[:], start=True, stop=True)
                    nc.vector.tensor_copy(
                        out=pk[:pt, j, 0:HCOLS].bitcast(bf16),
                        in_=h_ps[:pt, 0:P])
                    nc.scalar.copy(
                        out=pk[:pt, j, ACOL:ACOL + heads],
                        in_=h_ps[:pt, P:P + heads])
                    if nt is not None and nt < n_tiles:
                        # a_dst -> bf16 hi + lo correction
                        nc.vector.tensor_copy(out=a_sb[:pt, nt, 0:heads],
                                              in_=h_ps[:pt, P + heads:P + H2])
                        hi_f = p1w.tile([P, heads], f32, tag="hif")
                        nc.vector.tensor_copy(out=hi_f[:pt],
                                              in_=a_sb[:pt, nt, 0:heads])
                        lo_f = p1w.tile([P, heads], f32, tag="lof")
                        nc.vector.tensor_tensor(out=lo_f[:pt],
                                                in0=h_ps[:pt, P + heads:P + H2],
                                                in1=hi_f[:pt],
                                                op=ALU.subtract)
                        nc.vector.tensor_copy(out=a_sb[:pt, nt, heads:H2],
                                              in_=lo_f[:pt])

                # 4-tile batches, tile-major layout: xb[:, j, :] = tile nb*4+j
                for nb in range(n_full_batches):
                    r0 = nb * XB * P
                    xr = p1x.tile([P, XB, P], f32, tag="xr")
                    nc.sync.dma_start(
                        out=xr[:],
                        in_=x_in[r0:r0 + XB * P, :]
                        .rearrange("(k p) c -> p k c", p=P))
                    xb = p1w.tile([P, XB, P], bf16, tag="xb")
                    nc.vector.tensor_copy(out=xb[:].rearrange("p k c -> p (k c)"),
                                          in_=xr[:].rearrange("p k c -> p (k c)"))
                    pk = p1s.tile([P, XB, ROW], f32, tag="pk")
                    for j in range(XB):
                        do_tile(xb[:, j, :], pk, j, nb * XB + j, P)
                    r_id = r0 // RANGE
                    base = r_id * RANGE
                    nc.scalar.dma_start(
                        out=tabs[r_id][r0 - base:r0 - base + XB * P, :]
                        .rearrange("(k p) c -> p k c", p=P),
                        in_=pk[:])

                # tail tiles (per-tile)
                for nt in range(n_full_batches * XB, n_nt):
                    r0 = nt * P
                    r1 = min(r0 + P, n_nodes)
                    pt = r1 - r0
                    xr = p1x.tile([P, P], f32, tag="xr1")
                    nc.sync.dma_start(out=xr[:pt], in_=x_in[r0:r1, :])
                    xb = p1w.tile([P, P], bf16, tag="xb1")
                    nc.vector.tensor_copy(out=xb[:pt], in_=xr[:pt])
                    pk = p1s.tile([P, 1, ROW], f32, tag="pk1")
                    do_tile(xb[:pt], pk, 0, nt, pt)
                    r_id = r0 // RANGE
                    base = r_id * RANGE
                    nc.scalar.dma_start(out=tabs[r_id][r0 - base:r1 - base, :],
                                        in_=pk[:pt, 0, :])

            # ---------------- phase 2: edges ----------------
            gq = [0]
            with tc.tile_pool(name="aggps", bufs=tiles_per_sc, space="PSUM") as aggp, \
                 tc.tile_pool(name="trps", bufs=2, space="PSUM") as trp, \
                 tc.tile_pool(name="aeps", bufs=1, space="PSUM") as aep, \
                 tc.tile_pool(name="smps", bufs=1, space="PSUM") as smp:
                psums = {}
                done = {t: 0 for t in range(n_tiles)}
                for sc in range(n_sc):
                    t_lo = sc * tiles_per_sc
                    t_hi = min(t_lo + tiles_per_sc, n_tiles)
                    for t in range(t_lo, t_hi):
                        psums[t] = aggp.tile([P, NAGG], f32, tag="agg",
                                             name=f"agg{t}")
                    for r in range(n_ranges):
                        cap = int(caps[sc, r])
                        if cap == 0:
                            continue
                        K = cap // P
                        g = gbp.tile([P, K, ROW], f32, tag="g",
                                     name=f"g_{sc}_{r}")
                        off = 0
                        while off < cap:
                            piece = min(gather_piece, cap - off)
                            nc.gpsimd.dma_gather(
                                out_ap=g[:, off // P: (off + piece + P - 1) // P, :],
                                in_ap=tabs[r][:],
                                idxs_ap=gidx_sb[:, int(o16[sc, r]) + off // 16:
                                                int(o16[sc, r]) + (off + piece) // 16],
                                num_idxs=piece,
                                num_idxs_reg=piece,
                                elem_size=ROW,
                                queue_num=gq[0] % 4,
                            )
                            gq[0] += 1
                            off += piece

                        unit_pairs = [(k, t) for k in range(K)
                                      for t in pairs[(sc, r, k)]]
                        npair = len(unit_pairs)
                        c0 = colof[(sc, r) + unit_pairs[0]]
                        ae_ps = aep.tile([P, K * H2], f32, tag="ae",
                                         name=f"ae_{sc}_{r}")
                        # one batched is_equal for all pairs of the unit:
                        # mc_all[p, i, d] = (iota[d] == dl[p, c0+i])
                        mc_all = mcp.tile([P, npair, P], bf16, tag="mc",
                                          name=f"mc_{sc}_{r}")
                        iosl = iota_b[:]
                        gsl = gdl_sb[:, c0:c0 + npair]
                        nc.vector.tensor_tensor(
                            out=mc_all[:],
                            in0=bass.AP(tensor=iosl.tensor, offset=iosl.offset,
                                        ap=[iosl.ap[0], [0, npair], [1, P]]),
                            in1=bass.AP(tensor=gsl.tensor, offset=gsl.offset,
                                        ap=[gsl.ap[0], [1, npair], [0, P]]),
                            op=ALU.is_equal)
                        mc_t = {}
                        for pi, (k, t) in enumerate(unit_pairs):
                            mc_t[(k, t)] = mc_all[:, pi, :]
                        kfirst = {}
                        klast = {}
                        for (k, t) in unit_pairs:
                            kfirst.setdefault(k, (k, t))
                            klast[k] = (k, t)
                        for pb in range(0, npair, 4):
                            grpp = unit_pairs[pb:pb + 4]
                            tr_ps = trp.tile([P, 4 * P], bf16, tag="tr")
                            for i, (k, t) in enumerate(grpp):
                                nc.tensor.transpose(
                                    out=tr_ps[:, i * P:(i + 1) * P],
                                    in_=mc_t[(k, t)], identity=identb[:])
                            mct = mctp.tile([P, 4 * P], bf16, tag="mct")
                            nc.scalar.copy(out=mct[:, 0:len(grpp) * P],
                                           in_=tr_ps[:, 0:len(grpp) * P])
                            for i, (k, t) in enumerate(grpp):
                                nc.tensor.matmul(
                                    out=ae_ps[:, k * H2:(k + 1) * H2],
                                    lhsT=mct[:, i * P:(i + 1) * P],
                                    rhs=a_sb[:, t, :],
                                    start=kfirst[k] == (k, t),
                                    stop=klast[k] == (k, t))

                        # --- batched edge math over the unit ---
                        aesl = ae_ps[:]
                        s_t = spl.tile([P, K * heads], f32, tag="s")
                        nc.vector.tensor_tensor(
                            out=s_t[:].rearrange("p (k h) -> p k h", k=K),
                            in0=g[:, :, ACOL:ACOL + heads],
                            in1=bass.AP(tensor=aesl.tensor,
                                        offset=aesl.offset,
                                        ap=[aesl.ap[0], [H2, K], [1, heads]]),
                            op=ALU.add)
                        nc.vector.tensor_tensor(
                            out=s_t[:].rearrange("p (k h) -> p k h", k=K),
                            in0=s_t[:].rearrange("p (k h) -> p k h", k=K),
                            in1=bass.AP(tensor=aesl.tensor,
                                        offset=aesl.offset + heads,
                                        ap=[aesl.ap[0], [H2, K], [1, heads]]),
                            op=ALU.add)
                        lr = spl.tile([P, K * heads], f32, tag="lr")
                        nc.scalar.activation(out=lr[:], in_=s_t[:],
                                             func=AF.Prelu,
                                             alpha=alpha_c[:, 0:1])
                        me = mep.tile([P, K, NAGG], bf16, tag="me",
                                      name=f"me_{sc}_{r}")
                        nc.scalar.activation(
                            out=me[:, :, out_ch:NAGG],
                            in_=lr[:].rearrange("p (k h) -> p k h", k=K),
                            func=AF.Exp)
                        g2 = g[:, 0, 0:HCOLS].bitcast(bf16)   # [P, 128] bf16
                        for hd in range(heads):
                            exv = me[:, :, out_ch + hd:out_ch + hd + 1]
                            nc.vector.tensor_tensor(
                                out=me[:, :, hd * head_dim:(hd + 1) * head_dim],
                                in0=bass.AP(
                                    tensor=g2.tensor,
                                    offset=g2.offset + hd * head_dim,
                                    ap=[g2.ap[0], [2 * ROW, K], [1, head_dim]]),
                                in1=bass.AP(
                                    tensor=exv.tensor, offset=exv.offset,
                                    ap=[exv.ap[0], exv.ap[1], [0, head_dim]]),
                                op=ALU.mult)

                        # --- aggregation matmuls ---
                        for (k, t) in unit_pairs:
                            first = done[t] == 0
                            last = done[t] == len(tile_pairs[t]) - 1
                            nc.tensor.matmul(out=psums[t][:],
                                             lhsT=mc_t[(k, t)],
                                             rhs=me[:, k, :],
                                             start=first, stop=last)
                            done[t] += 1

                    # ---- per-tile epilogue (after all ranges of sc) ----
                    for t in range(t_lo, t_hi):
                        d0 = t * P
                        pt = min(d0 + P, shard) - d0
                        ps = psums[t]
                        den = epl.tile([P, heads], f32, tag="den")
                        nc.vector.tensor_scalar(
                            out=den[:pt], in0=ps[:pt, out_ch:NAGG],
                            scalar1=1e-16, scalar2=None, op0=ALU.add)
                        rden = epl.tile([P, heads], f32, tag="rden")
                        nc.vector.reciprocal(out=rden[:pt], in_=den[:pt])
                        rv = rden[:pt]
                        rd_b = bass.AP(tensor=rv.tensor, offset=rv.offset,
                                       ap=[rv.ap[0], rv.ap[1], [0, head_dim]])
                        nc.vector.tensor_tensor(
                            out=o_sto[:pt, t, :].rearrange(
                                "p (h c) -> p h c", h=heads),
                            in0=ps[:pt, 0:out_ch].rearrange(
                                "p (h c) -> p h c", h=heads),
                            in1=rd_b, op=ALU.mult)
                        st_in = epl.tile([P, 2 * out_ch], bf16, tag="stin")
                        if pt < P:
                            nc.vector.memset(st_in[:], 0.0)
                        nc.scalar.copy(out=st_in[:pt, 0:out_ch],
                                       in_=o_sto[:pt, t, :])
                        nc.vector.tensor_tensor(out=st_in[:pt, out_ch:],
                                                in0=o_sto[:pt, t, :],
                                                in1=o_sto[:pt, t, :],
                                                op=ALU.mult)
                        sm_ps = smp.tile([1, 2 * out_ch], f32, tag="sm")
                        nc.tensor.matmul(out=sm_ps[:], lhsT=ones_bf[:],
                                         rhs=st_in[:], start=True, stop=True)
                        nc.vector.tensor_tensor(out=stats_acc[:],
                                                in0=stats_acc[:], in1=sm_ps[:],
                                                op=ALU.add)

            _es.close()

            # ---------------- phase 2.5: BN stats allreduce ----------------
            with tc.tile_pool(name="bn", bufs=1) as bnp, \
                 tc.tile_pool(name="bnps", bufs=1, space="PSUM") as bnps:
                nc.sync.dma_start(out=bn_in[:], in_=stats_acc[:])
                nc.gpsimd.collective_compute(
                    "AllReduce", ALU.add,
                    replica_groups=[list(range(8))],
                    ins=[bn_in[:]], outs=[bn_out[:]],
                )
                tots = bnp.tile([1, 2 * out_ch], f32)
                nc.sync.dma_start(out=tots[:], in_=bn_out[:])
                gam = bnp.tile([1, out_ch], f32)
                nc.sync.dma_start(out=gam[:], in_=gamma_in[:])
                bet = bnp.tile([1, out_ch], f32)
                nc.sync.dma_start(out=bet[:], in_=beta_in[:])
                mu = bnp.tile([1, out_ch], f32)
                nc.vector.tensor_scalar(out=mu[:], in0=tots[:, 0:out_ch],
                                        scalar1=1.0 / n_nodes, scalar2=None,
                                        op0=ALU.mult)
                ex2 = bnp.tile([1, out_ch], f32)
                nc.vector.tensor_scalar(out=ex2[:], in0=tots[:, out_ch:],
                                        scalar1=1.0 / n_nodes, scalar2=None,
                                        op0=ALU.mult)
                musq = bnp.tile([1, out_ch], f32)
                nc.vector.tensor_tensor(out=musq[:], in0=mu[:], in1=mu[:],
                                        op=ALU.mult)
                var = bnp.tile([1, out_ch], f32)
                nc.vector.tensor_tensor(out=var[:], in0=ex2[:], in1=musq[:],
                                        op=ALU.subtract)
                sd = bnp.tile([1, out_ch], f32)
                nc.scalar.activation(out=sd[:], in_=var[:], func=AF.Sqrt,
                                     bias=eps_c[0:1, 0:1], scale=1.0)
                rs = bnp.tile([1, out_ch], f32)
                nc.vector.reciprocal(out=rs[:], in_=sd[:])
                scale_r = bnp.tile([1, out_ch], f32)
                nc.vector.tensor_tensor(out=scale_r[:], in0=rs[:], in1=gam[:],
                                        op=ALU.mult)
                mshift = bnp.tile([1, out_ch], f32)
                nc.vector.tensor_tensor(out=mshift[:], in0=mu[:], in1=scale_r[:],
                                        op=ALU.mult)
                shift_r = bnp.tile([1, out_ch], f32)
                nc.vector.tensor_tensor(out=shift_r[:], in0=bet[:], in1=mshift[:],
                                        op=ALU.subtract)
                scsh = bnp.tile([1, 2 * out_ch], f32r)
                nc.vector.tensor_copy(out=scsh[:, 0:out_ch], in_=scale_r[:])
                nc.vector.tensor_copy(out=scsh[:, out_ch:], in_=shift_r[:])
                bc_ps = bnps.tile([P, 2 * out_ch], f32)
                nc.tensor.matmul(out=bc_ps[:], lhsT=ones1r[:], rhs=scsh[:],
                                 start=True, stop=True)
                scale_bc = cs.tile([P, out_ch], f32)
                nc.vector.tensor_copy(out=scale_bc[:], in_=bc_ps[:, 0:out_ch])
                shift_bc = cs.tile([P, out_ch], f32)
                nc.scalar.copy(out=shift_bc[:], in_=bc_ps[:, out_ch:])

            # ---------------- phase 3: normalize (from SBUF) ----------------
            with tc.tile_pool(name="p3", bufs=3) as p3:
                for sc in range(n_sc):
                    t_lo = sc * tiles_per_sc
                    t_hi = min(t_lo + tiles_per_sc, n_tiles)
                    nbt = t_hi - t_lo
                    y_t = p3.tile([P, tiles_per_sc, out_ch], f32, tag="y3")
                    for t in range(t_lo, t_hi):
                        j = t - t_lo
                        nc.vector.tensor_tensor(out=y_t[:, j, :],
                                                in0=o_sto[:, t, :],
                                                in1=scale_bc[:], op=ALU.mult)
                        nc.vector.tensor_tensor(out=y_t[:, j, :],
                                                in0=y_t[:, j, :],
                                                in1=shift_bc[:], op=ALU.add)
                        nc.scalar.activation(out=y_t[:, j, :], in_=y_t[:, j, :],
                                             func=AF.Prelu,
                                             alpha=alpha_c[:, 0:1])
                    d0 = t_lo * P
                    d1 = min(t_hi * P, shard)
                    nfull = (d1 - d0) // P
                    if nfull:
                        nc.sync.dma_start(
                            out=out_fin[d0:d0 + nfull * P, :]
                            .rearrange("(k p) c -> p k c", p=P),
                            in_=y_t[:, 0:nfull, :])
                    rem = (d1 - d0) - nfull * P
                    if rem:
                        nc.sync.dma_start(
                            out=out_fin[d0 + nfull * P:d1, :],
                            in_=y_t[:rem, nfull, :])

    nc.compile()
    return nc


def run(x, edge_index, W, att_src, att_dst, bias, gamma, beta,
        n_cores=8, tiles_per_sc=4, trace=False, tmpdir=None):
    n_nodes, in_ch = x.shape
    heads, head_dim = att_src.shape
    out_ch = heads * head_dim
    shard = n_nodes // n_cores

    src = np.asarray(edge_index[0], np.int64)
    dst = np.asarray(edge_index[1], np.int64)
    sched, per_core = build_schedule(src, dst, n_nodes, n_cores, tiles_per_sc)
    nc = build_program(sched, heads, head_dim)

    x = np.asarray(x, np.float32)
    iota_b = np.tile(np.arange(P, dtype=np.float32), (P, 1)) \
        .astype(ml_dtypes.bfloat16)
    in_maps = []
    for c in range(n_cores):
        in_maps.append({
            "x": np.roll(x, -c * shard, axis=0),
            "w": np.asarray(W, np.float32),
            "att_src": np.asarray(att_src, np.float32),
            "att_dst": np.asarray(att_dst, np.float32),
            "gamma": np.asarray(gamma, np.float32).reshape(1, out_ch),
            "beta": (np.asarray(beta, np.float32)
                     + np.asarray(bias, np.float32) * 0.0).reshape(1, out_ch),
            "g_idx": per_core[c]["g_idx"],
            "g_dl": per_core[c]["g_dl"],
            "ident": np.eye(P, dtype=np.float32),
            "iota_b": iota_b,
        })
    res = run_bass_kernel_spmd(nc, in_maps, core_ids=list(range(n_cores)),
                               trace=trace, tmpdir=tmpdir)
    out = np.concatenate([res.results[c]["out"] for c in range(n_cores)], axis=0)
    return out, res


def _install_ntff_hook():
    """Best-effort NTFF profile hook for trace mode (missing in this image)."""
    import sys, types
    try:
        import antenv.axon_hooks  # noqa: F401
        return
    except ImportError:
        pass
    try:
        from trn_agent_boot.trn_boot import _ntff_profile_via_ctypes
        hook = _ntff_profile_via_ctypes('/opt/axon/libaxon_pjrt.so')
    except Exception:
        hook = None
    mod = types.ModuleType("antenv.axon_hooks")
    mod.get_axon_ntff_profile_hook = lambda: hook
    mod.set_axon_ntff_profile_hook = lambda h: None
    sys.modules["antenv.axon_hooks"] = mod


def kernel(**inputs):
    x = np.asarray(inputs["x"], np.float32)
    edge_index = np.asarray(inputs["edge_index"])
    W = np.asarray(inputs["W"], np.float32)
    att_src = np.asarray(inputs["att_src"], np.float32)
    att_dst = np.asarray(inputs["att_dst"], np.float32)
    bias = np.asarray(inputs["bias"], np.float32)
    gamma = np.asarray(inputs["gamma"], np.float32)
    beta = np.asarray(inputs["beta"], np.float32)
    import os
    trace = bool(os.environ.get("GAT_TRACE"))
    if trace:
        _install_ntff_hook()
    tmpdir = os.environ.get("GAT_TRACE_DIR")
    out, res = run(x, edge_index, W, att_src, att_dst, bias, gamma, beta,
                   trace=trace, tmpdir=tmpdir)
    kernel.last_result = res
    return out
